# revision 28
# baseline (speedup 1.0000x reference)
"""ChebNetII distributed Trainium2 kernel (8 NeuronCores).

Strategy:
  * Rows (nodes) sharded 12500/core. MLP computed on-device per core in
    bf16 (fp32 PSUM accumulation).
  * Chebyshev propagation in "z-space": z = D^-1/2 Tx, so the per-edge
    weight is exactly 1 (pure adjacency gather+sum) and the D scaling is a
    per-row multiply:  z_{k+1} = -2 deg^-1 * S(z_k) - z_{k-1},
    where S(z)[r] = sum_{edges (r,c)} z[c].
  * Per prop step the full z table (bf16, node pairs packed into 256B rows)
    is AllGathered into a SHARED-address-space DRAM table (one physical
    copy per chip; makes the collective ~free vs per-core Local copies).
  * Each core bulk dma_gathers its edges' source pairs (4 streams =
    2 index windows x 2 node parities, int16 index limit) in 512-descriptor
    calls round-robined over 4 SWDGE queues (512 descs = 32/engine keeps
    the descriptor ring un-congested; 1024-desc calls hit a throughput
    cliff), and reduces slots into rows with identity-matmul PSUM
    accumulation over a degree-sorted slot schedule. Partials are realigned
    back to canonical row order with small dma_gathers.
"""
import os
import sys
import time

sys.path.insert(0, "/opt/trn_rl_repo")

import numpy as np
import ml_dtypes

K_RUN = 10
STAGE = "full"
TRACE = False                                    # set by test.py for profiling
LAST = {}                                        # exec_time_ns etc. for test.py

N = 100000
K = 10
F = 64
NFEAT, NHID = 512, 256
NCORES = 8
RPC = 12500            # rows per core
RPC_PAD = 12544        # 98*128
NBLK = RPC_PAD // 128  # 98
PAIRS_PC = RPC_PAD // 2          # 6272
TBL_PAIRS = NCORES * PAIRS_PC    # 50176
WROWS = 6250           # real rows per half (window) per core
HALF_ROWS = 6272       # contrib rows per half (incl 22 zero-pad rows)
HALF_PAIRS = HALF_ROWS // 2      # 3136 pairs per half per core
WIN_PAIRS = NCORES * HALF_PAIRS  # 25088 pairs per window table
ZERO_IDX = 3125        # core 0's first zero-pad pair (same in both halves)
P = 128
BG = 4                 # row-blocks per psum group
GROUP = P * BG         # 512
NGROUPS = 25           # 12800 sorted rows
ROWS_SORT_PAD = NGROUPS * GROUP
GCH = 512              # idxs per main dma_gather call
SECPC = GCH // GROUP   # sections per gather call (2)
SLAB = 16              # idx sections per slab load
RC = 896               # realign chunk rows (=7*128); 12544/896 = 14 chunks
ACC_ROWS = 13056       # 12544 canonical + 512 dummy rows for sorted-pad slots


def _prep(edge_index):
    row = edge_index[0].astype(np.int64)
    col = edge_index[1].astype(np.int64)

    deg = np.bincount(row, minlength=N).astype(np.int64)

    # window = row-half within each core (A: rows 0..6249, B: 6250..12499)
    lr_col = col % RPC
    c_col = col // RPC
    w = (lr_col >= WROWS).astype(np.int64)
    lidx = (c_col * HALF_PAIRS + (lr_col - w * WROWS) // 2).astype(np.int64)
    par = col % 2
    s_of_e = 2 * w + par
    core = row // RPC
    lr = row % RPC

    key = (core * 4 + s_of_e) * RPC + lr
    order = np.argsort(key, kind="stable")
    core_s, s_s, lr_s, lidx_s = core[order], s_of_e[order], lr[order], lidx[order]
    kk = key[order]

    degs = np.bincount(kk, minlength=NCORES * 4 * RPC).reshape(NCORES, 4, RPC)

    pi = np.zeros((NCORES, 4, RPC), np.int64)
    inv_pi = np.zeros((NCORES, 4, RPC), np.int64)
    S_cs = np.zeros((NCORES, 4, NGROUPS), np.int64)
    for c in range(NCORES):
        for si in range(4):
            o = np.argsort(-degs[c, si], kind="stable")
            pi[c, si] = o
            inv_pi[c, si, o] = np.arange(RPC)
            d_pad = np.zeros(ROWS_SORT_PAD, np.int64)
            d_pad[:RPC] = degs[c, si, o]
            S_cs[c, si] = d_pad.reshape(NGROUPS, GROUP).max(1)
    S_sched = S_cs.max(axis=0)          # [4, NGROUPS]
    T_s = [int(GROUP * S_sched[si].sum()) for si in range(4)]
    cumS = [np.concatenate([[0], np.cumsum(S_sched[si])]) for si in range(4)]

    # slot position of each edge within its stream
    first = np.ones(len(kk), bool)
    first[1:] = kk[1:] != kk[:-1]
    seg_ids = np.cumsum(first) - 1
    starts = np.flatnonzero(first)
    m_in_row = np.arange(len(kk)) - starts[seg_ids]

    streams = [[np.full(T_s[si], ZERO_IDX, np.int16) for si in range(4)]
               for _ in range(NCORES)]
    for c in range(NCORES):
        msk_c = core_s == c
        for si in range(4):
            msk = msk_c & (s_s == si)
            pos = inv_pi[c, si, lr_s[msk]]
            g = pos // GROUP
            b = pos % GROUP
            off = GROUP * cumS[si][g] + GROUP * m_in_row[msk] + b
            streams[c][si][off] = lidx_s[msk].astype(np.int16)

    return degs, deg, pi, inv_pi, S_sched, streams, T_s


def _wrap_idx(idx_flat):
    """[n] -> [128, n/16] wrapped (i -> (i%16, i//16)) + replicated x8."""
    n = len(idx_flat)
    assert n % 16 == 0
    a = idx_flat.reshape(n // 16, 16).T  # [16, n/16]
    return np.ascontiguousarray(np.tile(a, (8, 1)))


def _build_program(S_sched, T_s, coe):
    import concourse.bass as bass
    import concourse.tile as tile
    from concourse import bacc, mybir
    from concourse.library_config import mlp as mlp_lib

    dt = mybir.dt
    Alu = mybir.AluOpType
    Act = mybir.ActivationFunctionType

    nc = bacc.Bacc("TRN2", target_bir_lowering=False, debug=False,
                   num_devices=NCORES, num_swdge_queues=4,
                   dynamic_dma_scratch_size=65536)

    lite = STAGE in ("nop2", "nop3")   # tiny inputs: measures pure exec/transfer floor
    if lite:
        featT = nc.dram_tensor("featT", [1, 1], dt.float32, kind="ExternalInput")
        idx_d = [nc.dram_tensor(f"idx{s}", [1, 1], dt.int16,
                                kind="ExternalInput") for s in range(4)]
        ridx_d = [nc.dram_tensor(f"ridx{s}", [1, 1], dt.int16,
                                 kind="ExternalInput") for s in range(4)]
    else:
        featT = nc.dram_tensor("featT", [NFEAT, RPC_PAD], dt.bfloat16, kind="ExternalInput")
        idx_d = [nc.dram_tensor(f"idx{s}", [P, T_s[s] // 16], dt.int16,
                                kind="ExternalInput") for s in range(4)]
        ridx_d = [nc.dram_tensor(f"ridx{s}", [P, RPC_PAD // 16], dt.int16,
                                 kind="ExternalInput") for s in range(4)]
    f16 = dt.float16
    W1T = nc.dram_tensor("W1T", [NFEAT, NHID], dt.float32, kind="ExternalInput")
    b1t_d = nc.dram_tensor("b1t", [P, 2], dt.float32, kind="ExternalInput")
    W2T = nc.dram_tensor("W2T", [NHID, F], dt.float32, kind="ExternalInput")
    b2t_d = nc.dram_tensor("b2t", [F, 1], dt.float32, kind="ExternalInput")
    dinv_d = nc.dram_tensor("dinv", [P, NBLK], dt.float32, kind="ExternalInput")
    n2d2_d = nc.dram_tensor("n2d2", [P, NBLK], dt.float32, kind="ExternalInput")
    sqd_d = nc.dram_tensor("sqd", [P, NBLK], dt.float32, kind="ExternalInput")
    if STAGE == "nop3":
        out_d = nc.dram_tensor("out", [1, F], dt.float32, kind="ExternalOutput")
    else:
        out_d = nc.dram_tensor("out", [RPC, F], dt.float32, kind="ExternalOutput")

    # section lists per stream: [(g, m, is_last)]
    sections = []
    for s in range(4):
        sec = []
        for g in range(NGROUPS):
            Sg = int(S_sched[s][g])
            for m in range(Sg):
                sec.append((g, m, m == Sg - 1))
        sections.append(sec)

    with tile.TileContext(nc) as tc:
        with (
            tc.tile_pool(name="dram", bufs=1, space="DRAM") as dram,
            tc.tile_pool(name="consts", bufs=1) as consts,
            tc.tile_pool(name="zs", bufs=1) as zs,
            tc.tile_pool(name="mlp", bufs=2) as mlppool,
            tc.tile_pool(name="gp", bufs=9) as gpool,
            tc.tile_pool(name="ip", bufs=4) as ipool,
            tc.tile_pool(name="ev", bufs=4) as evpool,
            tc.tile_pool(name="rt", bufs=4) as rtpool,
            tc.tile_pool(name="ps1", bufs=2, space="PSUM") as ps1,
            tc.tile_pool(name="ps2", bufs=1, space="PSUM") as ps2,
            tc.tile_pool(name="psT", bufs=1, space="PSUM") as psT,
            tc.tile_pool(name="psG", bufs=4, space="PSUM") as psG,
        ):
            nc.gpsimd.load_library(mlp_lib)

            contribs = [dram.tile([RPC_PAD, F], f16, name=f"contrib{k}")
                        for k in range(K)]
            tablesA = [dram.tile([WIN_PAIRS, 2 * F], f16,
                                 name=f"tableA{k}", addr_space="Shared")
                       for k in range(K)]
            tablesB = [dram.tile([WIN_PAIRS, 2 * F], f16,
                                 name=f"tableB{k}", addr_space="Shared")
                       for k in range(K)]
            partials = [dram.tile([ROWS_SORT_PAD, F], dt.float32, name=f"partial{s}")
                        for s in range(4)]

            # ---- constants ----
            iota_p = consts.tile([P, 1], dt.int32)
            nc.gpsimd.iota(iota_p[:], pattern=[[0, 1]], base=0, channel_multiplier=1)
            iota_pf = consts.tile([P, 1], dt.float32)
            nc.vector.tensor_copy(iota_pf[:], iota_p[:])
            iota_f = consts.tile([P, P], dt.int32)
            nc.gpsimd.iota(iota_f[:], pattern=[[1, P]], base=0, channel_multiplier=0)
            iota_ff = consts.tile([P, P], dt.float32)
            nc.vector.tensor_copy(iota_ff[:], iota_f[:])
            ident_bf = consts.tile([P, P], f16)
            nc.vector.tensor_tensor(out=ident_bf[:], in0=iota_ff[:],
                                    in1=iota_pf[:].to_broadcast([P, P]),
                                    op=Alu.is_equal)
            ident64 = consts.tile([F, F], dt.float32)
            nc.vector.tensor_tensor(out=ident64[:], in0=iota_ff[:F, :F],
                                    in1=iota_pf[:F, :].to_broadcast([F, F]),
                                    op=Alu.is_equal)

            w1 = consts.tile([P, 4, NHID], dt.float32)
            nc.sync.dma_start(w1[:], W1T[:, :].rearrange("(k p) h -> p k h", p=P))
            w2 = consts.tile([P, 2, F], dt.float32)
            nc.sync.dma_start(w2[:], W2T[:, :].rearrange("(k p) h -> p k h", p=P))
            w1b = consts.tile([P, 4, NHID], dt.bfloat16)
            nc.vector.tensor_copy(w1b[:], w1[:])
            w2b = consts.tile([P, 2, F], dt.bfloat16)
            nc.vector.tensor_copy(w2b[:], w2[:])
            b1tt = consts.tile([P, 2], dt.float32)
            nc.sync.dma_start(b1tt[:], b1t_d[:, :])
            b2tt = consts.tile([F, 1], dt.float32)
            nc.sync.dma_start(b2tt[:], b2t_d[:, :])
            dinv_t = consts.tile([P, NBLK], dt.float32)
            nc.sync.dma_start(dinv_t[:], dinv_d[:, :])
            n2d2_t = consts.tile([P, NBLK], dt.float32)
            nc.sync.dma_start(n2d2_t[:], n2d2_d[:, :])
            sqd_t = consts.tile([P, NBLK], dt.float32)
            nc.sync.dma_start(sqd_t[:], sqd_d[:, :])

            ridx_t = []
            for si in range(4):
                rtile = consts.tile([P, RPC_PAD // 16], dt.int16,
                                    name=f"ridx_t{si}")
                if STAGE not in ("nop2", "nop3"):
                    nc.sync.dma_start(rtile[:], ridx_d[si][:, :])
                ridx_t.append(rtile)

            # zero the contrib pad rows once (22 rows at the end of each
            # half: 6250..6271 and 12522..12543)
            zpad = consts.tile([22, F], f16)
            nc.vector.memset(zpad[:], 0.0)
            for k in range(K):
                nc.sync.dma_start(contribs[k][WROWS:HALF_ROWS, :], zpad[:])
                nc.sync.dma_start(
                    contribs[k][HALF_ROWS + WROWS:RPC_PAD, :], zpad[:])

            # ---- persistent state ----
            zA = zs.tile([P, NBLK, F], f16)
            zB = zs.tile([P, NBLK, F], f16)
            out_acc = zs.tile([P, NBLK, F], dt.float32)
            s_sum = zs.tile([P, NBLK, F], dt.float32)

            # ---- MLP -> z0 (into zA) ----
            chunks = [(i * 512, 512) for i in range(24)] + [(24 * 512, 256)]
            if STAGE in ("nop", "nop2", "nop3"):
                nc.vector.memset(zA[:], 0.0)
                chunks = []
            for (c0, C) in chunks:
                ft = mlppool.tile([P, 4, 512], dt.bfloat16, tag="featT", bufs=2)
                nc.sync.dma_start(
                    ft[:, :, :C],
                    featT[:, c0:c0 + C].rearrange("(k p) c -> p k c", p=P))
                x1h = []
                for h in range(2):
                    pm = ps1.tile([P, 512], dt.float32, space="PSUM", tag="ps1")
                    for k in range(4):
                        nc.tensor.matmul(out=pm[:, :C],
                                         lhsT=w1b[:, k, 128 * h:128 * (h + 1)],
                                         rhs=ft[:, k, :C],
                                         start=(k == 0), stop=(k == 3))
                    xh = mlppool.tile([P, 512], dt.bfloat16, tag="x1")
                    nc.scalar.activation(xh[:, :C], pm[:, :C], Act.Relu,
                                         bias=b1tt[:, h:h + 1])
                    x1h.append(xh)
                pm2 = ps2.tile([F, 512], dt.float32, space="PSUM", tag="ps2")
                for h in range(2):
                    nc.tensor.matmul(out=pm2[:, :C], lhsT=w2b[:, h, :],
                                     rhs=x1h[h][:, :C],
                                     start=(h == 0), stop=(h == 1))
                x2 = mlppool.tile([F, 512], dt.float32, tag="x2")
                nc.scalar.activation(x2[:, :C], pm2[:, :C], Act.Identity,
                                     bias=b2tt[:, 0:1])
                for jj in range(C // 128):
                    jb = c0 // 128 + jj
                    pt = psT.tile([P, F], dt.float32, space="PSUM", tag="psT")
                    nc.tensor.transpose(pt[:], x2[:, 128 * jj:128 * (jj + 1)],
                                        ident64[:])
                    nc.vector.tensor_tensor(
                        out=zA[:, jb, :], in0=pt[:],
                        in1=dinv_t[:, jb:jb + 1].to_broadcast([P, F]),
                        op=Alu.mult)

            # out_acc = coe0/2 * z0
            nc.vector.tensor_scalar_mul(out_acc[:], zA[:], float(coe[0]) / 2.0)

            HB = HALF_ROWS // P  # 49 blocks = rows 0..6271 (half A)

            def pub_half(zsrc, h, kk):
                contrib = contribs[kk]
                if h == 0:
                    # z rows 0..6249 -> contrib 0..6249
                    nc.sync.dma_start(
                        contrib[0:6144, :].rearrange("(j p) f -> p j f", p=P),
                        zsrc[:, 0:48, :])
                    nc.sync.dma_start(contrib[6144:WROWS, :],
                                      zsrc[0:106, 48, :])
                else:
                    # z rows 6250..12499 -> contrib 6272..12521
                    nc.sync.dma_start(contrib[HALF_ROWS:6294, :],
                                      zsrc[106:128, 48, :])
                    nc.sync.dma_start(
                        contrib[6294:12438, :]
                        .rearrange("(j p) f -> p j f", p=P),
                        zsrc[:, 49:97, :])
                    nc.sync.dma_start(contrib[12438:12522, :],
                                      zsrc[0:84, 97, :])

            def cc_half(h, kk):
                if STAGE in ("pub", "gonocc", "fullnocc"):
                    return
                contrib = contribs[kk]
                if h == 0:
                    nc.gpsimd.collective_compute(
                        "AllGather", Alu.bypass,
                        replica_groups=[list(range(NCORES))],
                        ins=[contrib[0:HALF_ROWS, :].opt()],
                        outs=[tablesA[kk][:].opt()])
                else:
                    nc.gpsimd.collective_compute(
                        "AllGather", Alu.bypass,
                        replica_groups=[list(range(NCORES))],
                        ins=[contrib[HALF_ROWS:RPC_PAD, :].opt()],
                        outs=[tablesB[kk][:].opt()])

            abl = STAGE in ("mlp", "pub", "gather", "gatheronly", "gonocc")
            if STAGE not in ("mlponly", "nop", "nop2", "nop3") and not abl:
                # publish z0 right after the MLP (both halves)
                pub_half(zA, 0, 0)
                cc_half(0, 0)
                pub_half(zA, 1, 0)
                cc_half(1, 0)

            z_prev, z_cur = zA, zB
            for k in range(1, K_RUN + 1):
                if STAGE in ("mlponly", "nop", "nop2", "nop3"):
                    continue
                if abl:
                    # ablation stages: publish z0 at top of each step
                    pub_half(zA, 0, k - 1)
                    cc_half(0, k - 1)
                    pub_half(zA, 1, k - 1)
                    cc_half(1, k - 1)
                if STAGE in ("pub", "mlp"):
                    continue
                # gather + identity-matmul reduce, per stream.
                # 512-desc dma_gather calls (ring stays <=32 descs/engine),
                # round-robin over 4 SWDGE queues; TWO gathers share one idx
                # load + one gt tile + one PE matmul over 1024 slots.
                # interleave the 4 streams round-robin, one SWDGE queue per
                # stream: all 4 queues stay loaded with in-order chains.
                # Each dma_gather covers SECPC consecutive sections (1024
                # idxs = the HW per-call descriptor ceiling).
                psum_maps = [{} for _ in range(4)]
                islabs = [None] * 4
                qcnt = 0
                for sgroup in ((0, 1), (2, 3)):
                    ngc = {s: (len(sections[s]) + SECPC - 1) // SECPC
                           for s in sgroup}
                    for cix in range(max(ngc.values())):
                        for s in sgroup:
                            if cix >= ngc[s]:
                                continue
                            t0 = cix * SECPC
                            nsec = min(SECPC, len(sections[s]) - t0)
                            par = s % 2
                            src = (tablesA[k - 1][:, :] if s < 2
                                   else tablesB[k - 1][:, :])
                            i0 = t0 * GROUP
                            sl = t0 % SLAB
                            if sl == 0:
                                ns = min(SLAB, len(sections[s]) - t0)
                                islabs[s] = ipool.tile(
                                    [P, SLAB * GROUP // 16], dt.int16,
                                    tag=f"idxslab{s}", name=f"islab{s}")
                                nc.sync.dma_start(
                                    islabs[s][:, :ns * GROUP // 16],
                                    idx_d[s][:, i0 // 16:
                                             (i0 + ns * GROUP) // 16])
                            it = islabs[s][:, sl * GROUP // 16:
                                           (sl + nsec) * GROUP // 16]
                            gt = gpool.tile([P, SECPC * BG, 2 * F], f16,
                                            tag="g")
                            nc.gpsimd.dma_gather(
                                gt[:, :nsec * BG, :], src, it, nsec * GROUP,
                                nsec * GROUP, 2 * F, elem_step=2 * F,
                                queue_num=qcnt % 4)
                            qcnt += 1
                            for j in range(nsec):
                                g, m, last = sections[s][t0 + j]
                                if m == 0:
                                    psum_maps[s][g] = psG.tile(
                                        [P, BG, F], dt.float32,
                                        space="PSUM", tag="psG",
                                        name=f"pg_{k}_{s}_{g}")
                                pm = psum_maps[s][g]
                                nc.tensor.matmul(
                                    out=pm[:],
                                    lhsT=ident_bf[:],
                                    rhs=gt[:, j * BG:(j + 1) * BG,
                                           par * F:(par + 1) * F],
                                    start=(m == 0), stop=last)
                                if last:
                                    ev = evpool.tile([P, BG, F], dt.float32,
                                                     tag="ev")
                                    nc.vector.tensor_copy(ev[:], pm[:])
                                    nc.sync.dma_start(
                                        partials[s][GROUP * g:GROUP * (g + 1), :]
                                        .rearrange("(b p) f -> p b f", p=P),
                                        ev[:])

                if STAGE in ("gather", "gatheronly", "gonocc"):
                    continue
                # realign + combine + publish, split into row halves so the
                # half-A collective overlaps half-B realign/combine and the
                # half-B collective overlaps the next step's half-A gathers.
                z_new = z_cur if k == 1 else z_prev
                rchunks = []
                r0 = 0
                while r0 < RPC_PAD:
                    rchunks.append((r0, min(RC, RPC_PAD - r0)))
                    r0 += RC
                nhalf = len(rchunks) // 2
                for h in (0, 1):
                    for rc, (r0, rn) in list(enumerate(rchunks))[
                            h * nhalf:(h + 1) * nhalf]:
                        for s in range(4):
                            reg = max(0, min(RPC - r0, rn))
                            rt = rtpool.tile([P, RC // P, F], dt.float32,
                                             tag="rt")
                            nc.gpsimd.dma_gather(
                                rt[:, :rn // P, :], partials[s][:, :],
                                ridx_t[s][:, r0 // 16:(r0 + rn) // 16],
                                rn, reg, F,
                                elem_step=F, queue_num=(s + rc) % 4)
                            dst = s_sum[:, r0 // P:(r0 + rn) // P, :]
                            if s == 0:
                                nc.vector.tensor_copy(dst, rt[:, :rn // P, :])
                            else:
                                nc.vector.tensor_tensor(
                                    out=dst, in0=dst,
                                    in1=rt[:, :rn // P, :], op=Alu.add)

                    j0, j1 = (0, HB) if h == 0 else (HB, NBLK)
                    if k == 1:
                        # z1 = 0.5 * n2d2 * S
                        for j in range(j0, j1):
                            nc.vector.tensor_tensor(
                                out=z_cur[:, j, :], in0=s_sum[:, j, :],
                                in1=n2d2_t[:, j:j + 1].to_broadcast([P, F]),
                                op=Alu.mult)
                        nc.vector.tensor_scalar_mul(
                            z_cur[:, j0:j1, :], z_cur[:, j0:j1, :], 0.5)
                    else:
                        # z_next = n2d2*S - z_prev  (write into z_prev slot)
                        for j in range(j0, j1):
                            nc.vector.tensor_tensor(
                                out=s_sum[:, j, :], in0=s_sum[:, j, :],
                                in1=n2d2_t[:, j:j + 1].to_broadcast([P, F]),
                                op=Alu.mult)
                        nc.vector.tensor_tensor(
                            out=z_new[:, j0:j1, :], in0=s_sum[:, j0:j1, :],
                            in1=z_new[:, j0:j1, :], op=Alu.subtract)
                    if k < K_RUN:
                        pub_half(z_new, h, k)
                        cc_half(h, k)
                    # out_acc += coe[k] * z_new  (reuse s_sum as scratch)
                    nc.vector.tensor_scalar_mul(
                        s_sum[:, j0:j1, :], z_new[:, j0:j1, :], float(coe[k]))
                    nc.vector.tensor_tensor(
                        out=out_acc[:, j0:j1, :], in0=out_acc[:, j0:j1, :],
                        in1=s_sum[:, j0:j1, :], op=Alu.add)
                if k > 1:
                    z_prev, z_cur = z_cur, z_new

            # final scale by sqrt(deg) and store
            for j in range(NBLK):
                nc.vector.tensor_tensor(
                    out=out_acc[:, j, :], in0=out_acc[:, j, :],
                    in1=sqd_t[:, j:j + 1].to_broadcast([P, F]), op=Alu.mult)
            if STAGE == "nop3":
                nc.sync.dma_start(out_d[0:1, :], out_acc[0:1, 0, :])
            else:
                nc.sync.dma_start(
                    out_d[0:12416, :].rearrange("(j p) f -> p j f", p=P),
                    out_acc[:, 0:97, :])
                nc.sync.dma_start(
                    out_d[12416:RPC, :].rearrange("(j p) f -> p j f", p=84),
                    out_acc[0:84, 97:98, :])

    t0 = time.time()
    nc.compile()
    print(f"bacc compile: {time.time() - t0:.1f}s", flush=True)
    return nc


def prepare(inputs):
    """Host preprocessing + program build. Returns (nc, in_maps)."""
    feature = np.asarray(inputs["feature"], np.float32)
    W1 = np.asarray(inputs["W1"], np.float32)
    b1 = np.asarray(inputs["b1"], np.float32)
    W2 = np.asarray(inputs["W2"], np.float32)
    b2 = np.asarray(inputs["b2"], np.float32)
    temp = np.asarray(inputs["temp"], np.float32)
    edge_index = np.asarray(inputs["edge_index"])

    # Chebyshev coefficients (host, tiny)
    coe_tmp = np.maximum(temp, 0.0)
    j = np.arange(K + 1, dtype=np.float64)
    theta = (K - j + 0.5) * np.pi / (K + 1)
    i = np.arange(K + 1, dtype=np.float64)
    T = np.cos(i[:, None] * theta[None, :])
    coe = ((2.0 / (K + 1)) * (T @ coe_tmp.astype(np.float64))).astype(np.float32)

    degs, deg, pi, inv_pi, S_sched, streams, T_s = _prep(edge_index)

    degf = deg.astype(np.float32)
    dinv = np.where(deg > 0, 1.0 / np.sqrt(np.maximum(degf, 1.0)), 1.0).astype(np.float32)
    n2d2 = np.where(deg > 0, -2.0 / np.maximum(degf, 1.0), -2.0).astype(np.float32)
    sqd = np.where(deg > 0, np.sqrt(np.maximum(degf, 1.0)), 1.0).astype(np.float32)

    def rowmajor(vec_c):  # [RPC] -> [128, NBLK] with r = 128*j + p
        v = np.zeros(RPC_PAD, np.float32)
        v[:RPC] = vec_c
        return np.ascontiguousarray(v.reshape(NBLK, P).T)

    in_maps = []
    for c in range(NCORES):
        fT = np.zeros((NFEAT, RPC_PAD), ml_dtypes.bfloat16)
        fT[:, :RPC] = feature[c * RPC:(c + 1) * RPC].T.astype(ml_dtypes.bfloat16)
        m = {
            "featT": fT,
            "W1T": np.ascontiguousarray(W1.T),
            "b1t": np.ascontiguousarray(b1.reshape(2, P).T),
            "W2T": np.ascontiguousarray(W2.T),
            "b2t": np.ascontiguousarray(b2.reshape(F, 1)),
            "dinv": rowmajor(dinv[c * RPC:(c + 1) * RPC]),
            "n2d2": rowmajor(n2d2[c * RPC:(c + 1) * RPC]),
            "sqd": rowmajor(sqd[c * RPC:(c + 1) * RPC]),
        }
        for s in range(4):
            m[f"idx{s}"] = _wrap_idx(streams[c][s])
            r = np.full(RPC_PAD, -1, np.int16)
            r[:RPC] = inv_pi[c, s]
            m[f"ridx{s}"] = _wrap_idx(r)
        if STAGE in ("nop2", "nop3"):
            m["featT"] = np.zeros((1, 1), ml_dtypes.bfloat16)
            for s in range(4):
                m[f"idx{s}"] = np.zeros((1, 1), np.int16)
                m[f"ridx{s}"] = np.zeros((1, 1), np.int16)
        in_maps.append(m)

    t0 = time.time()
    nc = _build_program(S_sched, T_s, coe)
    print(f"build+compile total: {time.time() - t0:.1f}s", flush=True)
    return nc, in_maps


def assemble(results):
    out = np.empty((N, F), np.float32)
    for c in range(NCORES):
        out[c * RPC:(c + 1) * RPC] = results[c]["out"]
    return out


def kernel(**inputs):
    from concourse.bass_utils import run_bass_kernel_spmd

    nc, in_maps = prepare(inputs)
    t0 = time.time()
    res = run_bass_kernel_spmd(nc, in_maps, list(range(NCORES)), trace=TRACE)
    print(f"neff compile+run: {time.time() - t0:.1f}s", flush=True)
    LAST["exec_time_ns"] = res.exec_time_ns
    LAST["profile_json"] = res.profile_json
    return assemble(res.results)



# revision 34
# speedup vs baseline: 1.3162x; 1.3162x over previous
"""ChebNetII distributed Trainium2 kernel (8 NeuronCores).

Strategy:
  * Rows (nodes) sharded 12500/core. MLP computed on-device per core in
    bf16 (fp32 PSUM accumulation).
  * Chebyshev propagation in "z-space": z = D^-1/2 Tx, so the per-edge
    weight is exactly 1 (pure adjacency gather+sum) and the D scaling is a
    per-row multiply:  z_{k+1} = -2 deg^-1 * S(z_k) - z_{k-1},
    where S(z)[r] = sum_{edges (r,c)} z[c].
  * Per prop step the full z table (bf16, node pairs packed into 256B rows)
    is AllGathered into a SHARED-address-space DRAM table (one physical
    copy per chip; makes the collective ~free vs per-core Local copies).
  * Each core bulk dma_gathers its edges' source pairs (4 streams =
    2 index windows x 2 node parities, int16 index limit) in 512-descriptor
    calls round-robined over 4 SWDGE queues (512 descs = 32/engine keeps
    the descriptor ring un-congested; 1024-desc calls hit a throughput
    cliff), and reduces slots into rows with identity-matmul PSUM
    accumulation over a degree-sorted slot schedule. Partials are realigned
    back to canonical row order with small dma_gathers.
"""
import os
import sys
import time

sys.path.insert(0, "/opt/trn_rl_repo")

import numpy as np
import ml_dtypes

K_RUN = 10
STAGE = "full"
TRACE = False                                    # set by test.py for profiling
LAST = {}                                        # exec_time_ns etc. for test.py

N = 100000
K = 10
F = 64
NFEAT, NHID = 512, 256
NCORES = 8
RPC = 12500            # rows per core
RPC_PAD = 12544        # 98*128
NBLK = RPC_PAD // 128  # 98
PAIRS_PC = RPC_PAD // 2          # 6272
TBL_PAIRS = NCORES * PAIRS_PC    # 50176
WIN_PAIRS = TBL_PAIRS // 2       # 25088
ZERO_IDX = 6250        # local pair idx of a guaranteed-zero pair (first pad pair of window's first core block)
P = 128
BG = 4                 # row-blocks per psum group
GROUP = P * BG         # 512
NGROUPS = 25           # 12800 sorted rows
ROWS_SORT_PAD = NGROUPS * GROUP
GCH = 512              # idxs per main dma_gather call (SWDGE ring is ~256 descs/engine)
SLAB = 16              # idx sections per slab load
RC = 896               # combine chunk rows (=7*128); 12544/896 = 14 chunks
ACC_ROWS = 13056       # 12544 canonical + 512 dummy rows for sorted-pad slots


def _prep(edge_index):
    row = edge_index[0].astype(np.int64)
    col = edge_index[1].astype(np.int64)

    deg = np.bincount(row, minlength=N).astype(np.int64)

    q_t = PAIRS_PC * (col // RPC) + (col % RPC) // 2
    w = q_t // WIN_PAIRS
    lidx = (q_t % WIN_PAIRS).astype(np.int64)
    par = col % 2
    s_of_e = 2 * w + par
    core = row // RPC
    lr = row % RPC

    key = (core * 4 + s_of_e) * RPC + lr
    order = np.argsort(key, kind="stable")
    core_s, s_s, lr_s, lidx_s = core[order], s_of_e[order], lr[order], lidx[order]
    kk = key[order]

    degs = np.bincount(kk, minlength=NCORES * 4 * RPC).reshape(NCORES, 4, RPC)

    pi = np.zeros((NCORES, 4, RPC), np.int64)
    inv_pi = np.zeros((NCORES, 4, RPC), np.int64)
    S_cs = np.zeros((NCORES, 4, NGROUPS), np.int64)
    for c in range(NCORES):
        for si in range(4):
            o = np.argsort(-degs[c, si], kind="stable")
            pi[c, si] = o
            inv_pi[c, si, o] = np.arange(RPC)
            d_pad = np.zeros(ROWS_SORT_PAD, np.int64)
            d_pad[:RPC] = degs[c, si, o]
            S_cs[c, si] = d_pad.reshape(NGROUPS, GROUP).max(1)
    S_sched = S_cs.max(axis=0)          # [4, NGROUPS]
    T_s = [int(GROUP * S_sched[si].sum()) for si in range(4)]
    cumS = [np.concatenate([[0], np.cumsum(S_sched[si])]) for si in range(4)]

    # slot position of each edge within its stream
    first = np.ones(len(kk), bool)
    first[1:] = kk[1:] != kk[:-1]
    seg_ids = np.cumsum(first) - 1
    starts = np.flatnonzero(first)
    m_in_row = np.arange(len(kk)) - starts[seg_ids]

    streams = [[np.full(T_s[si], ZERO_IDX, np.int16) for si in range(4)]
               for _ in range(NCORES)]
    for c in range(NCORES):
        msk_c = core_s == c
        for si in range(4):
            msk = msk_c & (s_s == si)
            pos = inv_pi[c, si, lr_s[msk]]
            g = pos // GROUP
            b = pos % GROUP
            off = GROUP * cumS[si][g] + GROUP * m_in_row[msk] + b
            streams[c][si][off] = lidx_s[msk].astype(np.int16)

    return degs, deg, pi, inv_pi, S_sched, streams, T_s


def _wrap_idx(idx_flat):
    """[n] -> [128, n/16] wrapped (i -> (i%16, i//16)) + replicated x8."""
    n = len(idx_flat)
    assert n % 16 == 0
    a = idx_flat.reshape(n // 16, 16).T  # [16, n/16]
    return np.ascontiguousarray(np.tile(a, (8, 1)))


def _build_program(S_sched, T_s, coe):
    import concourse.bass as bass
    import concourse.tile as tile
    from concourse import bacc, mybir
    from concourse.library_config import mlp as mlp_lib

    dt = mybir.dt
    Alu = mybir.AluOpType
    Act = mybir.ActivationFunctionType

    nc = bacc.Bacc("TRN2", target_bir_lowering=False, debug=False,
                   num_devices=NCORES, num_swdge_queues=4,
                   dynamic_dma_scratch_size=32768)

    lite = STAGE in ("nop2", "nop3")   # tiny inputs: measures pure exec/transfer floor
    if lite:
        featT = nc.dram_tensor("featT", [1, 1], dt.float32, kind="ExternalInput")
        idx_d = [nc.dram_tensor(f"idx{s}", [1, 1], dt.int16,
                                kind="ExternalInput") for s in range(4)]
        ridx_d = [nc.dram_tensor(f"ridx{s}", [1, 1], dt.int16,
                                 kind="ExternalInput") for s in range(4)]
    else:
        featT = nc.dram_tensor("featT", [NFEAT, RPC_PAD], dt.bfloat16, kind="ExternalInput")
        idx_d = [nc.dram_tensor(f"idx{s}", [P, T_s[s] // 16], dt.int16,
                                kind="ExternalInput") for s in range(4)]
        ridx_d = [nc.dram_tensor(f"ridx{s}", [P, RPC_PAD // 16], dt.int16,
                                 kind="ExternalInput") for s in range(4)]
    W1T = nc.dram_tensor("W1T", [NFEAT, NHID], dt.float32, kind="ExternalInput")
    b1t_d = nc.dram_tensor("b1t", [P, 2], dt.float32, kind="ExternalInput")
    W2T = nc.dram_tensor("W2T", [NHID, F], dt.float32, kind="ExternalInput")
    b2t_d = nc.dram_tensor("b2t", [F, 1], dt.float32, kind="ExternalInput")
    dinv_d = nc.dram_tensor("dinv", [P, NBLK], dt.float32, kind="ExternalInput")
    n2d2_d = nc.dram_tensor("n2d2", [P, NBLK], dt.float32, kind="ExternalInput")
    sqd_d = nc.dram_tensor("sqd", [P, NBLK], dt.float32, kind="ExternalInput")
    if STAGE == "nop3":
        out_d = nc.dram_tensor("out", [1, F], dt.float32, kind="ExternalOutput")
    else:
        out_d = nc.dram_tensor("out", [RPC, F], dt.float32, kind="ExternalOutput")

    # section lists per stream: [(g, m, is_last)]
    sections = []
    for s in range(4):
        sec = []
        for g in range(NGROUPS):
            Sg = int(S_sched[s][g])
            for m in range(Sg):
                sec.append((g, m, m == Sg - 1))
        sections.append(sec)

    with tile.TileContext(nc) as tc:
        with (
            tc.tile_pool(name="dram", bufs=1, space="DRAM") as dram,
            tc.tile_pool(name="consts", bufs=1) as consts,
            tc.tile_pool(name="zs", bufs=1) as zs,
            tc.tile_pool(name="mlp", bufs=2) as mlppool,
            tc.tile_pool(name="gp", bufs=12) as gpool,
            tc.tile_pool(name="ip", bufs=6) as ipool,
            tc.tile_pool(name="ev", bufs=4) as evpool,
            tc.tile_pool(name="rt", bufs=4) as rtpool,
            tc.tile_pool(name="ps1", bufs=2, space="PSUM") as ps1,
            tc.tile_pool(name="ps2", bufs=1, space="PSUM") as ps2,
            tc.tile_pool(name="psT", bufs=1, space="PSUM") as psT,
            tc.tile_pool(name="psG", bufs=4, space="PSUM") as psG,
        ):
            nc.gpsimd.load_library(mlp_lib)

            contribs = [dram.tile([RPC_PAD, F], dt.bfloat16, name=f"contrib{k}")
                        for k in range(K)]
            tables = [dram.tile([TBL_PAIRS, 2 * F], dt.bfloat16,
                                name=f"table{k}", addr_space="Shared")
                      for k in range(K)]
            halftables = [dram.tile([TBL_PAIRS // 2, 2 * F], dt.bfloat16,
                                    name=f"htable{k}", addr_space="Shared")
                          for k in range(K)] if STAGE == "gohalfcc" else None
            partials = [dram.tile([ROWS_SORT_PAD, F], dt.float32, name=f"partial{s}")
                        for s in range(4)]

            # ---- constants ----
            iota_p = consts.tile([P, 1], dt.int32)
            nc.gpsimd.iota(iota_p[:], pattern=[[0, 1]], base=0, channel_multiplier=1)
            iota_pf = consts.tile([P, 1], dt.float32)
            nc.vector.tensor_copy(iota_pf[:], iota_p[:])
            iota_f = consts.tile([P, P], dt.int32)
            nc.gpsimd.iota(iota_f[:], pattern=[[1, P]], base=0, channel_multiplier=0)
            iota_ff = consts.tile([P, P], dt.float32)
            nc.vector.tensor_copy(iota_ff[:], iota_f[:])
            ident_bf = consts.tile([P, P], dt.bfloat16)
            nc.vector.tensor_tensor(out=ident_bf[:], in0=iota_ff[:],
                                    in1=iota_pf[:].to_broadcast([P, P]),
                                    op=Alu.is_equal)
            ident64 = consts.tile([F, F], dt.float32)
            nc.vector.tensor_tensor(out=ident64[:], in0=iota_ff[:F, :F],
                                    in1=iota_pf[:F, :].to_broadcast([F, F]),
                                    op=Alu.is_equal)

            w1 = consts.tile([P, 4, NHID], dt.float32)
            nc.sync.dma_start(w1[:], W1T[:, :].rearrange("(k p) h -> p k h", p=P))
            w2 = consts.tile([P, 2, F], dt.float32)
            nc.sync.dma_start(w2[:], W2T[:, :].rearrange("(k p) h -> p k h", p=P))
            w1b = consts.tile([P, 4, NHID], dt.bfloat16)
            nc.vector.tensor_copy(w1b[:], w1[:])
            w2b = consts.tile([P, 2, F], dt.bfloat16)
            nc.vector.tensor_copy(w2b[:], w2[:])
            b1tt = consts.tile([P, 2], dt.float32)
            nc.sync.dma_start(b1tt[:], b1t_d[:, :])
            b2tt = consts.tile([F, 1], dt.float32)
            nc.sync.dma_start(b2tt[:], b2t_d[:, :])
            dinv_t = consts.tile([P, NBLK], dt.float32)
            nc.sync.dma_start(dinv_t[:], dinv_d[:, :])
            n2d2_t = consts.tile([P, NBLK], dt.float32)
            nc.sync.dma_start(n2d2_t[:], n2d2_d[:, :])
            sqd_t = consts.tile([P, NBLK], dt.float32)
            nc.sync.dma_start(sqd_t[:], sqd_d[:, :])

            ridx_t = []
            for si in range(4):
                rtile = consts.tile([P, RPC_PAD // 16], dt.int16,
                                    name=f"ridx_t{si}")
                if STAGE not in ("nop2", "nop3"):
                    nc.sync.dma_start(rtile[:], ridx_d[si][:, :])
                ridx_t.append(rtile)

            # zero the contrib pad rows once (rows RPC..RPC_PAD)
            zpad = consts.tile([44, F], dt.bfloat16)
            nc.vector.memset(zpad[:], 0.0)
            for k in range(K):
                nc.sync.dma_start(contribs[k][RPC:RPC_PAD, :], zpad[:])

            # ---- persistent state ----
            zA = zs.tile([P, NBLK, F], dt.float32)
            zB = zs.tile([P, NBLK, F], dt.float32)
            out_acc = zs.tile([P, NBLK, F], dt.float32)
            s_sum = zs.tile([P, NBLK, F], dt.float32)
            zpub = zs.tile([P, NBLK, F], dt.bfloat16)

            # ---- MLP -> z0 (into zA) ----
            chunks = [(i * 512, 512) for i in range(24)] + [(24 * 512, 256)]
            if STAGE in ("nop", "nop2", "nop3"):
                nc.vector.memset(zA[:], 0.0)
                chunks = []
            for (c0, C) in chunks:
                ft = mlppool.tile([P, 4, 512], dt.bfloat16, tag="featT", bufs=2)
                nc.sync.dma_start(
                    ft[:, :, :C],
                    featT[:, c0:c0 + C].rearrange("(k p) c -> p k c", p=P))
                x1h = []
                for h in range(2):
                    pm = ps1.tile([P, 512], dt.float32, space="PSUM", tag="ps1")
                    for k in range(4):
                        nc.tensor.matmul(out=pm[:, :C],
                                         lhsT=w1b[:, k, 128 * h:128 * (h + 1)],
                                         rhs=ft[:, k, :C],
                                         start=(k == 0), stop=(k == 3))
                    xh = mlppool.tile([P, 512], dt.bfloat16, tag="x1")
                    nc.scalar.activation(xh[:, :C], pm[:, :C], Act.Relu,
                                         bias=b1tt[:, h:h + 1])
                    x1h.append(xh)
                pm2 = ps2.tile([F, 512], dt.float32, space="PSUM", tag="ps2")
                for h in range(2):
                    nc.tensor.matmul(out=pm2[:, :C], lhsT=w2b[:, h, :],
                                     rhs=x1h[h][:, :C],
                                     start=(h == 0), stop=(h == 1))
                x2 = mlppool.tile([F, 512], dt.float32, tag="x2")
                nc.scalar.activation(x2[:, :C], pm2[:, :C], Act.Identity,
                                     bias=b2tt[:, 0:1])
                for jj in range(C // 128):
                    jb = c0 // 128 + jj
                    pt = psT.tile([P, F], dt.float32, space="PSUM", tag="psT")
                    nc.tensor.transpose(pt[:], x2[:, 128 * jj:128 * (jj + 1)],
                                        ident64[:])
                    nc.vector.tensor_tensor(
                        out=zA[:, jb, :], in0=pt[:],
                        in1=dinv_t[:, jb:jb + 1].to_broadcast([P, F]),
                        op=Alu.mult)

            # out_acc = coe0/2 * z0
            nc.vector.tensor_scalar_mul(out_acc[:], zA[:], float(coe[0]) / 2.0)

            z_prev, z_cur = zA, zB
            for k in range(1, K_RUN + 1):
                if STAGE in ("mlponly", "nop", "nop2", "nop3"):
                    continue
                zsrc = z_prev if (k == 1 or STAGE in ("mlp", "pub", "gather", "gatheronly", "gonocc", "gohalfcc")) else z_cur
                # publish z_{k-1}: cast-DMA into contrib then AllGather
                contrib = contribs[k - 1]
                table = tables[k - 1]
                nc.vector.tensor_copy(zpub[:], zsrc[:])
                nc.sync.dma_start(
                    contrib[0:12416, :].rearrange("(j p) f -> p j f", p=P),
                    zpub[:, 0:97, :])
                nc.sync.dma_start(
                    contrib[12416:RPC, :].rearrange("(j p) f -> p j f", p=84),
                    zpub[0:84, 97:98, :])
                if STAGE == "pub":
                    continue
                if STAGE == "gonocc":
                    pass
                elif STAGE == "gohalfcc":
                    nc.gpsimd.collective_compute(
                        "AllGather", Alu.bypass,
                        replica_groups=[list(range(NCORES))],
                        ins=[contrib[0:RPC_PAD // 2, :].opt()],
                        outs=[halftables[k - 1][:].opt()])
                else:
                    nc.gpsimd.collective_compute(
                        "AllGather", Alu.bypass,
                        replica_groups=[list(range(NCORES))],
                        ins=[contrib[:].opt()], outs=[table[:].opt()])

                if STAGE == "mlp":
                    continue
                # gather + identity-matmul reduce, per stream.
                # 512-desc dma_gather calls (ring stays <=32 descs/engine),
                # round-robin over 4 SWDGE queues; TWO gathers share one idx
                # load + one gt tile + one PE matmul over 1024 slots.
                qcnt = 0
                # interleave the 4 streams round-robin, one SWDGE queue per
                # stream: all 4 queues stay loaded with in-order chains
                psum_maps = [{} for _ in range(4)]
                islabs = [None] * 4
                maxsec = max(len(sections[s]) for s in range(4))
                for t in range(maxsec):
                    for s in range(4):
                        if t >= len(sections[s]):
                            continue
                        g, m, last = sections[s][t]
                        win = s // 2
                        par = s % 2
                        src = table[win * WIN_PAIRS:(win + 1) * WIN_PAIRS, :]
                        i0 = t * GROUP
                        sl = t % SLAB
                        if sl == 0:
                            ns = min(SLAB, len(sections[s]) - t)
                            islabs[s] = ipool.tile([P, SLAB * GROUP // 16],
                                                   dt.int16, tag=f"idxslab{s}",
                                                   name=f"islab{s}")
                            nc.sync.dma_start(
                                islabs[s][:, :ns * GROUP // 16],
                                idx_d[s][:, i0 // 16:(i0 + ns * GROUP) // 16])
                        it = islabs[s][:, sl * GROUP // 16:(sl + 1) * GROUP // 16]
                        gt = gpool.tile([P, BG, 2 * F], dt.bfloat16, tag="g")
                        nc.gpsimd.dma_gather(
                            gt[:], src, it, GROUP, GROUP, 2 * F,
                            elem_step=2 * F, queue_num=s)
                        if m == 0:
                            psum_maps[s][g] = psG.tile([P, BG, F], dt.float32,
                                                       space="PSUM", tag="psG",
                                                       name=f"pg_{k}_{s}_{g}")
                        pm = psum_maps[s][g]
                        nc.tensor.matmul(
                            out=pm[:],
                            lhsT=ident_bf[:],
                            rhs=gt[:, :, par * F:(par + 1) * F],
                            start=(m == 0), stop=last)
                        if last:
                            ev = evpool.tile([P, BG, F], dt.float32, tag="ev")
                            nc.vector.tensor_copy(ev[:], pm[:])
                            nc.sync.dma_start(
                                partials[s][GROUP * g:GROUP * (g + 1), :]
                                .rearrange("(b p) f -> p b f", p=P),
                                ev[:])

                if STAGE in ("gather", "gatheronly", "gonocc", "gohalfcc"):
                    continue
                # realign partials into s_sum
                for rc in range(RPC_PAD // RC):
                    for s in range(4):
                        reg = max(0, min(RPC - rc * RC, RC))
                        rt = rtpool.tile([P, RC // P, F], dt.float32, tag="rt")
                        nc.gpsimd.dma_gather(
                            rt[:], partials[s][:, :],
                            ridx_t[s][:, rc * (RC // 16):(rc + 1) * (RC // 16)],
                            RC, reg, F,
                            elem_step=F, queue_num=(s + rc) % 4)
                        dst = s_sum[:, rc * (RC // P):(rc + 1) * (RC // P), :]
                        if s == 0:
                            nc.vector.tensor_copy(dst, rt[:])
                        else:
                            nc.vector.tensor_tensor(out=dst, in0=dst, in1=rt[:],
                                                    op=Alu.add)

                # combine (per-block broadcast of -2/deg)
                if k == 1:
                    # z1 = -dinv2 * S = 0.5 * n2d2 * S
                    for j in range(NBLK):
                        nc.vector.tensor_tensor(
                            out=z_cur[:, j, :], in0=s_sum[:, j, :],
                            in1=n2d2_t[:, j:j + 1].to_broadcast([P, F]),
                            op=Alu.mult)
                    nc.vector.tensor_scalar_mul(z_cur[:], z_cur[:], 0.5)
                    z_new = z_cur
                else:
                    # z_next = n2d2*S - z_prev  (write into z_prev slot)
                    for j in range(NBLK):
                        nc.vector.tensor_tensor(
                            out=s_sum[:, j, :], in0=s_sum[:, j, :],
                            in1=n2d2_t[:, j:j + 1].to_broadcast([P, F]),
                            op=Alu.mult)
                    nc.vector.tensor_tensor(out=z_prev[:], in0=s_sum[:],
                                            in1=z_prev[:], op=Alu.subtract)
                    z_new = z_prev
                    z_prev, z_cur = z_cur, z_new
                # out_acc += coe[k] * z_new   (reuse s_sum as scratch)
                nc.vector.tensor_scalar_mul(s_sum[:], z_new[:], float(coe[k]))
                nc.vector.tensor_tensor(out=out_acc[:], in0=out_acc[:],
                                        in1=s_sum[:], op=Alu.add)

            # final scale by sqrt(deg) and store
            for j in range(NBLK):
                nc.vector.tensor_tensor(
                    out=out_acc[:, j, :], in0=out_acc[:, j, :],
                    in1=sqd_t[:, j:j + 1].to_broadcast([P, F]), op=Alu.mult)
            if STAGE == "nop3":
                nc.sync.dma_start(out_d[0:1, :], out_acc[0:1, 0, :])
            else:
                nc.sync.dma_start(
                    out_d[0:12416, :].rearrange("(j p) f -> p j f", p=P),
                    out_acc[:, 0:97, :])
                nc.sync.dma_start(
                    out_d[12416:RPC, :].rearrange("(j p) f -> p j f", p=84),
                    out_acc[0:84, 97:98, :])

    t0 = time.time()
    nc.compile()
    print(f"bacc compile: {time.time() - t0:.1f}s", flush=True)
    return nc


def prepare(inputs):
    """Host preprocessing + program build. Returns (nc, in_maps)."""
    feature = np.asarray(inputs["feature"], np.float32)
    W1 = np.asarray(inputs["W1"], np.float32)
    b1 = np.asarray(inputs["b1"], np.float32)
    W2 = np.asarray(inputs["W2"], np.float32)
    b2 = np.asarray(inputs["b2"], np.float32)
    temp = np.asarray(inputs["temp"], np.float32)
    edge_index = np.asarray(inputs["edge_index"])

    # Chebyshev coefficients (host, tiny)
    coe_tmp = np.maximum(temp, 0.0)
    j = np.arange(K + 1, dtype=np.float64)
    theta = (K - j + 0.5) * np.pi / (K + 1)
    i = np.arange(K + 1, dtype=np.float64)
    T = np.cos(i[:, None] * theta[None, :])
    coe = ((2.0 / (K + 1)) * (T @ coe_tmp.astype(np.float64))).astype(np.float32)

    degs, deg, pi, inv_pi, S_sched, streams, T_s = _prep(edge_index)

    degf = deg.astype(np.float32)
    dinv = np.where(deg > 0, 1.0 / np.sqrt(np.maximum(degf, 1.0)), 1.0).astype(np.float32)
    n2d2 = np.where(deg > 0, -2.0 / np.maximum(degf, 1.0), -2.0).astype(np.float32)
    sqd = np.where(deg > 0, np.sqrt(np.maximum(degf, 1.0)), 1.0).astype(np.float32)

    def rowmajor(vec_c):  # [RPC] -> [128, NBLK] with r = 128*j + p
        v = np.zeros(RPC_PAD, np.float32)
        v[:RPC] = vec_c
        return np.ascontiguousarray(v.reshape(NBLK, P).T)

    in_maps = []
    for c in range(NCORES):
        fT = np.zeros((NFEAT, RPC_PAD), ml_dtypes.bfloat16)
        fT[:, :RPC] = feature[c * RPC:(c + 1) * RPC].T.astype(ml_dtypes.bfloat16)
        m = {
            "featT": fT,
            "W1T": np.ascontiguousarray(W1.T),
            "b1t": np.ascontiguousarray(b1.reshape(2, P).T),
            "W2T": np.ascontiguousarray(W2.T),
            "b2t": np.ascontiguousarray(b2.reshape(F, 1)),
            "dinv": rowmajor(dinv[c * RPC:(c + 1) * RPC]),
            "n2d2": rowmajor(n2d2[c * RPC:(c + 1) * RPC]),
            "sqd": rowmajor(sqd[c * RPC:(c + 1) * RPC]),
        }
        for s in range(4):
            m[f"idx{s}"] = _wrap_idx(streams[c][s])
            r = np.full(RPC_PAD, -1, np.int16)
            r[:RPC] = inv_pi[c, s]
            m[f"ridx{s}"] = _wrap_idx(r)
        if STAGE in ("nop2", "nop3"):
            m["featT"] = np.zeros((1, 1), ml_dtypes.bfloat16)
            for s in range(4):
                m[f"idx{s}"] = np.zeros((1, 1), np.int16)
                m[f"ridx{s}"] = np.zeros((1, 1), np.int16)
        in_maps.append(m)

    t0 = time.time()
    nc = _build_program(S_sched, T_s, coe)
    print(f"build+compile total: {time.time() - t0:.1f}s", flush=True)
    return nc, in_maps


def assemble(results):
    out = np.empty((N, F), np.float32)
    for c in range(NCORES):
        out[c * RPC:(c + 1) * RPC] = results[c]["out"]
    return out


def kernel(**inputs):
    from concourse.bass_utils import run_bass_kernel_spmd

    nc, in_maps = prepare(inputs)
    t0 = time.time()
    res = run_bass_kernel_spmd(nc, in_maps, list(range(NCORES)), trace=TRACE)
    print(f"neff compile+run: {time.time() - t0:.1f}s", flush=True)
    LAST["exec_time_ns"] = res.exec_time_ns
    LAST["profile_json"] = res.profile_json
    return assemble(res.results)



# revision 35
# speedup vs baseline: 1.3641x; 1.0364x over previous
"""ChebNetII distributed Trainium2 kernel (8 NeuronCores).

Strategy:
  * Rows (nodes) sharded 12500/core. MLP computed on-device per core in
    bf16 (fp32 PSUM accumulation).
  * Chebyshev propagation in "z-space": z = D^-1/2 Tx, so the per-edge
    weight is exactly 1 (pure adjacency gather+sum) and the D scaling is a
    per-row multiply:  z_{k+1} = -2 deg^-1 * S(z_k) - z_{k-1},
    where S(z)[r] = sum_{edges (r,c)} z[c].
  * Per prop step the full z table (bf16, node pairs packed into 256B rows)
    is AllGathered into a SHARED-address-space DRAM table (one physical
    copy per chip; makes the collective ~free vs per-core Local copies).
  * Each core bulk dma_gathers its edges' source pairs (4 streams =
    2 index windows x 2 node parities, int16 index limit) in 512-descriptor
    calls round-robined over 4 SWDGE queues (512 descs = 32/engine keeps
    the descriptor ring un-congested; 1024-desc calls hit a throughput
    cliff), and reduces slots into rows with identity-matmul PSUM
    accumulation over a degree-sorted slot schedule. Partials are realigned
    back to canonical row order with small dma_gathers.
"""
import os
import sys
import time

sys.path.insert(0, "/opt/trn_rl_repo")

import numpy as np
import ml_dtypes

K_RUN = 10
STAGE = "full"
TRACE = False                                    # set by test.py for profiling
LAST = {}                                        # exec_time_ns etc. for test.py

N = 100000
K = 10
F = 64
NFEAT, NHID = 512, 256
NCORES = 8
RPC = 12500            # rows per core
RPC_PAD = 12544        # 98*128
NBLK = RPC_PAD // 128  # 98
PAIRS_PC = RPC_PAD // 2          # 6272
TBL_PAIRS = NCORES * PAIRS_PC    # 50176
WIN_PAIRS = TBL_PAIRS // 2       # 25088
ZERO_IDX = 6250        # local pair idx of a guaranteed-zero pair (first pad pair of window's first core block)
P = 128
BG = 4                 # row-blocks per psum group
GROUP = P * BG         # 512
NGROUPS = 25           # 12800 sorted rows
ROWS_SORT_PAD = NGROUPS * GROUP
GCH = 512              # idxs per main dma_gather call (SWDGE ring is ~256 descs/engine)
SLAB = 16              # idx sections per slab load
RC = 896               # combine chunk rows (=7*128); 12544/896 = 14 chunks
ACC_ROWS = 13056       # 12544 canonical + 512 dummy rows for sorted-pad slots


def _prep(edge_index):
    row = edge_index[0].astype(np.int64)
    col = edge_index[1].astype(np.int64)

    deg = np.bincount(row, minlength=N).astype(np.int64)

    q_t = PAIRS_PC * (col // RPC) + (col % RPC) // 2
    w = q_t // WIN_PAIRS
    lidx = (q_t % WIN_PAIRS).astype(np.int64)
    par = col % 2
    s_of_e = 2 * w + par
    core = row // RPC
    lr = row % RPC

    key = (core * 4 + s_of_e) * RPC + lr
    order = np.argsort(key, kind="stable")
    core_s, s_s, lr_s, lidx_s = core[order], s_of_e[order], lr[order], lidx[order]
    kk = key[order]

    degs = np.bincount(kk, minlength=NCORES * 4 * RPC).reshape(NCORES, 4, RPC)

    pi = np.zeros((NCORES, 4, RPC), np.int64)
    inv_pi = np.zeros((NCORES, 4, RPC), np.int64)
    S_cs = np.zeros((NCORES, 4, NGROUPS), np.int64)
    for c in range(NCORES):
        for si in range(4):
            o = np.argsort(-degs[c, si], kind="stable")
            pi[c, si] = o
            inv_pi[c, si, o] = np.arange(RPC)
            d_pad = np.zeros(ROWS_SORT_PAD, np.int64)
            d_pad[:RPC] = degs[c, si, o]
            S_cs[c, si] = d_pad.reshape(NGROUPS, GROUP).max(1)
    S_sched = S_cs.max(axis=0)          # [4, NGROUPS]
    T_s = [int(GROUP * S_sched[si].sum()) for si in range(4)]
    cumS = [np.concatenate([[0], np.cumsum(S_sched[si])]) for si in range(4)]

    # slot position of each edge within its stream
    first = np.ones(len(kk), bool)
    first[1:] = kk[1:] != kk[:-1]
    seg_ids = np.cumsum(first) - 1
    starts = np.flatnonzero(first)
    m_in_row = np.arange(len(kk)) - starts[seg_ids]

    streams = [[np.full(T_s[si], ZERO_IDX, np.int16) for si in range(4)]
               for _ in range(NCORES)]
    for c in range(NCORES):
        msk_c = core_s == c
        for si in range(4):
            msk = msk_c & (s_s == si)
            pos = inv_pi[c, si, lr_s[msk]]
            g = pos // GROUP
            b = pos % GROUP
            off = GROUP * cumS[si][g] + GROUP * m_in_row[msk] + b
            streams[c][si][off] = lidx_s[msk].astype(np.int16)

    return degs, deg, pi, inv_pi, S_sched, streams, T_s


def _wrap_idx(idx_flat):
    """[n] -> [128, n/16] wrapped (i -> (i%16, i//16)) + replicated x8."""
    n = len(idx_flat)
    assert n % 16 == 0
    a = idx_flat.reshape(n // 16, 16).T  # [16, n/16]
    return np.ascontiguousarray(np.tile(a, (8, 1)))


def _build_program(S_sched, T_s, coe):
    import concourse.bass as bass
    import concourse.tile as tile
    from concourse import bacc, mybir
    from concourse.library_config import mlp as mlp_lib

    dt = mybir.dt
    Alu = mybir.AluOpType
    Act = mybir.ActivationFunctionType

    nc = bacc.Bacc("TRN2", target_bir_lowering=False, debug=False,
                   num_devices=NCORES, num_swdge_queues=4,
                   dynamic_dma_scratch_size=32768)

    lite = STAGE in ("nop2", "nop3")   # tiny inputs: measures pure exec/transfer floor
    if lite:
        featT = nc.dram_tensor("featT", [1, 1], dt.float32, kind="ExternalInput")
        idx_d = [nc.dram_tensor(f"idx{s}", [1, 1], dt.int16,
                                kind="ExternalInput") for s in range(4)]
        ridx_d = [nc.dram_tensor(f"ridx{s}", [1, 1], dt.int16,
                                 kind="ExternalInput") for s in range(4)]
    else:
        featT = nc.dram_tensor("featT", [NFEAT, RPC_PAD], dt.bfloat16, kind="ExternalInput")
        idx_d = [nc.dram_tensor(f"idx{s}", [P, T_s[s] // 16], dt.int16,
                                kind="ExternalInput") for s in range(4)]
        ridx_d = [nc.dram_tensor(f"ridx{s}", [P, RPC_PAD // 16], dt.int16,
                                 kind="ExternalInput") for s in range(4)]
    f16 = dt.float16
    W1T = nc.dram_tensor("W1T", [NFEAT, NHID], dt.float32, kind="ExternalInput")
    b1t_d = nc.dram_tensor("b1t", [P, 2], dt.float32, kind="ExternalInput")
    W2T = nc.dram_tensor("W2T", [NHID, F], dt.float32, kind="ExternalInput")
    b2t_d = nc.dram_tensor("b2t", [F, 1], dt.float32, kind="ExternalInput")
    dinv_d = nc.dram_tensor("dinv", [P, NBLK], dt.float32, kind="ExternalInput")
    n2d2_d = nc.dram_tensor("n2d2", [P, NBLK], dt.float32, kind="ExternalInput")
    sqd_d = nc.dram_tensor("sqd", [P, NBLK], dt.float32, kind="ExternalInput")
    if STAGE == "nop3":
        out_d = nc.dram_tensor("out", [1, F], dt.float32, kind="ExternalOutput")
    else:
        out_d = nc.dram_tensor("out", [RPC, F], dt.float32, kind="ExternalOutput")

    # section lists per stream: [(g, m, is_last)]
    sections = []
    for s in range(4):
        sec = []
        for g in range(NGROUPS):
            Sg = int(S_sched[s][g])
            for m in range(Sg):
                sec.append((g, m, m == Sg - 1))
        sections.append(sec)

    with tile.TileContext(nc) as tc:
        with (
            tc.tile_pool(name="dram", bufs=1, space="DRAM") as dram,
            tc.tile_pool(name="consts", bufs=1) as consts,
            tc.tile_pool(name="zs", bufs=1) as zs,
            tc.tile_pool(name="mlp", bufs=2) as mlppool,
            tc.tile_pool(name="gp", bufs=16) as gpool,
            tc.tile_pool(name="ip", bufs=8) as ipool,
            tc.tile_pool(name="ev", bufs=4) as evpool,
            tc.tile_pool(name="rt", bufs=4) as rtpool,
            tc.tile_pool(name="ps1", bufs=2, space="PSUM") as ps1,
            tc.tile_pool(name="ps2", bufs=1, space="PSUM") as ps2,
            tc.tile_pool(name="psT", bufs=1, space="PSUM") as psT,
            tc.tile_pool(name="psG", bufs=4, space="PSUM") as psG,
        ):
            nc.gpsimd.load_library(mlp_lib)

            contribs = [dram.tile([RPC_PAD, F], f16, name=f"contrib{k}")
                        for k in range(K)]
            tables = [dram.tile([TBL_PAIRS, 2 * F], f16,
                                name=f"table{k}", addr_space="Shared")
                      for k in range(K)]
            halftables = [dram.tile([TBL_PAIRS // 2, 2 * F], f16,
                                    name=f"htable{k}", addr_space="Shared")
                          for k in range(K)] if STAGE == "gohalfcc" else None
            partials = [dram.tile([ROWS_SORT_PAD, F], dt.float32, name=f"partial{s}")
                        for s in range(4)]

            # ---- constants ----
            iota_p = consts.tile([P, 1], dt.int32)
            nc.gpsimd.iota(iota_p[:], pattern=[[0, 1]], base=0, channel_multiplier=1)
            iota_pf = consts.tile([P, 1], dt.float32)
            nc.vector.tensor_copy(iota_pf[:], iota_p[:])
            iota_f = consts.tile([P, P], dt.int32)
            nc.gpsimd.iota(iota_f[:], pattern=[[1, P]], base=0, channel_multiplier=0)
            iota_ff = consts.tile([P, P], dt.float32)
            nc.vector.tensor_copy(iota_ff[:], iota_f[:])
            ident_bf = consts.tile([P, P], f16)
            nc.vector.tensor_tensor(out=ident_bf[:], in0=iota_ff[:],
                                    in1=iota_pf[:].to_broadcast([P, P]),
                                    op=Alu.is_equal)
            ident64 = consts.tile([F, F], dt.float32)
            nc.vector.tensor_tensor(out=ident64[:], in0=iota_ff[:F, :F],
                                    in1=iota_pf[:F, :].to_broadcast([F, F]),
                                    op=Alu.is_equal)

            w1 = consts.tile([P, 4, NHID], dt.float32)
            nc.sync.dma_start(w1[:], W1T[:, :].rearrange("(k p) h -> p k h", p=P))
            w2 = consts.tile([P, 2, F], dt.float32)
            nc.sync.dma_start(w2[:], W2T[:, :].rearrange("(k p) h -> p k h", p=P))
            w1b = consts.tile([P, 4, NHID], dt.bfloat16)
            nc.vector.tensor_copy(w1b[:], w1[:])
            w2b = consts.tile([P, 2, F], dt.bfloat16)
            nc.vector.tensor_copy(w2b[:], w2[:])
            b1tt = consts.tile([P, 2], dt.float32)
            nc.sync.dma_start(b1tt[:], b1t_d[:, :])
            b2tt = consts.tile([F, 1], dt.float32)
            nc.sync.dma_start(b2tt[:], b2t_d[:, :])
            dinv_t = consts.tile([P, NBLK], dt.float32)
            nc.sync.dma_start(dinv_t[:], dinv_d[:, :])
            n2d2_t = consts.tile([P, NBLK], dt.float32)
            nc.sync.dma_start(n2d2_t[:], n2d2_d[:, :])
            sqd_t = consts.tile([P, NBLK], dt.float32)
            nc.sync.dma_start(sqd_t[:], sqd_d[:, :])

            ridx_t = []
            for si in range(4):
                rtile = consts.tile([P, RPC_PAD // 16], dt.int16,
                                    name=f"ridx_t{si}")
                if STAGE not in ("nop2", "nop3"):
                    nc.sync.dma_start(rtile[:], ridx_d[si][:, :])
                ridx_t.append(rtile)

            # zero the contrib pad rows once (rows RPC..RPC_PAD)
            zpad = consts.tile([44, F], f16)
            nc.vector.memset(zpad[:], 0.0)
            for k in range(K):
                nc.sync.dma_start(contribs[k][RPC:RPC_PAD, :], zpad[:])

            # ---- persistent state ----
            zA = zs.tile([P, NBLK, F], f16)
            zB = zs.tile([P, NBLK, F], f16)
            out_acc = zs.tile([P, NBLK, F], dt.float32)
            s_sum = zs.tile([P, NBLK, F], dt.float32)

            # ---- MLP -> z0 (into zA) ----
            chunks = [(i * 512, 512) for i in range(24)] + [(24 * 512, 256)]
            if STAGE in ("nop", "nop2", "nop3"):
                nc.vector.memset(zA[:], 0.0)
                chunks = []
            for (c0, C) in chunks:
                ft = mlppool.tile([P, 4, 512], dt.bfloat16, tag="featT", bufs=2)
                nc.sync.dma_start(
                    ft[:, :, :C],
                    featT[:, c0:c0 + C].rearrange("(k p) c -> p k c", p=P))
                x1h = []
                for h in range(2):
                    pm = ps1.tile([P, 512], dt.float32, space="PSUM", tag="ps1")
                    for k in range(4):
                        nc.tensor.matmul(out=pm[:, :C],
                                         lhsT=w1b[:, k, 128 * h:128 * (h + 1)],
                                         rhs=ft[:, k, :C],
                                         start=(k == 0), stop=(k == 3))
                    xh = mlppool.tile([P, 512], dt.bfloat16, tag="x1")
                    nc.scalar.activation(xh[:, :C], pm[:, :C], Act.Relu,
                                         bias=b1tt[:, h:h + 1])
                    x1h.append(xh)
                pm2 = ps2.tile([F, 512], dt.float32, space="PSUM", tag="ps2")
                for h in range(2):
                    nc.tensor.matmul(out=pm2[:, :C], lhsT=w2b[:, h, :],
                                     rhs=x1h[h][:, :C],
                                     start=(h == 0), stop=(h == 1))
                x2 = mlppool.tile([F, 512], dt.float32, tag="x2")
                nc.scalar.activation(x2[:, :C], pm2[:, :C], Act.Identity,
                                     bias=b2tt[:, 0:1])
                for jj in range(C // 128):
                    jb = c0 // 128 + jj
                    pt = psT.tile([P, F], dt.float32, space="PSUM", tag="psT")
                    nc.tensor.transpose(pt[:], x2[:, 128 * jj:128 * (jj + 1)],
                                        ident64[:])
                    nc.vector.tensor_tensor(
                        out=zA[:, jb, :], in0=pt[:],
                        in1=dinv_t[:, jb:jb + 1].to_broadcast([P, F]),
                        op=Alu.mult)

            # out_acc = coe0/2 * z0
            nc.vector.tensor_scalar_mul(out_acc[:], zA[:], float(coe[0]) / 2.0)

            z_prev, z_cur = zA, zB
            for k in range(1, K_RUN + 1):
                if STAGE in ("mlponly", "nop", "nop2", "nop3"):
                    continue
                zsrc = z_prev if (k == 1 or STAGE in ("mlp", "pub", "gather", "gatheronly", "gonocc", "gohalfcc")) else z_cur
                # publish z_{k-1}: cast-DMA into contrib then AllGather
                contrib = contribs[k - 1]
                table = tables[k - 1]
                nc.sync.dma_start(
                    contrib[0:12416, :].rearrange("(j p) f -> p j f", p=P),
                    zsrc[:, 0:97, :])
                nc.sync.dma_start(
                    contrib[12416:RPC, :].rearrange("(j p) f -> p j f", p=84),
                    zsrc[0:84, 97:98, :])
                if STAGE == "pub":
                    continue
                if STAGE == "gonocc":
                    pass
                elif STAGE == "gohalfcc":
                    nc.gpsimd.collective_compute(
                        "AllGather", Alu.bypass,
                        replica_groups=[list(range(NCORES))],
                        ins=[contrib[0:RPC_PAD // 2, :].opt()],
                        outs=[halftables[k - 1][:].opt()])
                else:
                    nc.gpsimd.collective_compute(
                        "AllGather", Alu.bypass,
                        replica_groups=[list(range(NCORES))],
                        ins=[contrib[:].opt()], outs=[table[:].opt()])

                if STAGE == "mlp":
                    continue
                # gather + identity-matmul reduce, per stream.
                # 512-desc dma_gather calls (ring stays <=32 descs/engine),
                # round-robin over 4 SWDGE queues; TWO gathers share one idx
                # load + one gt tile + one PE matmul over 1024 slots.
                qcnt = 0
                # interleave the 4 streams round-robin, one SWDGE queue per
                # stream: all 4 queues stay loaded with in-order chains
                psum_maps = [{} for _ in range(4)]
                islabs = [None] * 4
                maxsec = max(len(sections[s]) for s in range(4))
                for t in range(maxsec):
                    for s in range(4):
                        if t >= len(sections[s]):
                            continue
                        g, m, last = sections[s][t]
                        win = s // 2
                        par = s % 2
                        src = table[win * WIN_PAIRS:(win + 1) * WIN_PAIRS, :]
                        i0 = t * GROUP
                        sl = t % SLAB
                        if sl == 0:
                            ns = min(SLAB, len(sections[s]) - t)
                            islabs[s] = ipool.tile([P, SLAB * GROUP // 16],
                                                   dt.int16, tag=f"idxslab{s}",
                                                   name=f"islab{s}")
                            nc.sync.dma_start(
                                islabs[s][:, :ns * GROUP // 16],
                                idx_d[s][:, i0 // 16:(i0 + ns * GROUP) // 16])
                        it = islabs[s][:, sl * GROUP // 16:(sl + 1) * GROUP // 16]
                        gt = gpool.tile([P, BG, 2 * F], f16, tag="g")
                        nc.gpsimd.dma_gather(
                            gt[:], src, it, GROUP, GROUP, 2 * F,
                            elem_step=2 * F, queue_num=s)
                        if m == 0:
                            psum_maps[s][g] = psG.tile([P, BG, F], dt.float32,
                                                       space="PSUM", tag="psG",
                                                       name=f"pg_{k}_{s}_{g}")
                        pm = psum_maps[s][g]
                        nc.tensor.matmul(
                            out=pm[:],
                            lhsT=ident_bf[:],
                            rhs=gt[:, :, par * F:(par + 1) * F],
                            start=(m == 0), stop=last)
                        if last:
                            ev = evpool.tile([P, BG, F], dt.float32, tag="ev")
                            nc.vector.tensor_copy(ev[:], pm[:])
                            nc.sync.dma_start(
                                partials[s][GROUP * g:GROUP * (g + 1), :]
                                .rearrange("(b p) f -> p b f", p=P),
                                ev[:])

                if STAGE in ("gather", "gatheronly", "gonocc", "gohalfcc"):
                    continue
                # realign partials into s_sum
                for rc in range(RPC_PAD // RC):
                    for s in range(4):
                        reg = max(0, min(RPC - rc * RC, RC))
                        rt = rtpool.tile([P, RC // P, F], dt.float32, tag="rt")
                        nc.gpsimd.dma_gather(
                            rt[:], partials[s][:, :],
                            ridx_t[s][:, rc * (RC // 16):(rc + 1) * (RC // 16)],
                            RC, reg, F,
                            elem_step=F, queue_num=(s + rc) % 4)
                        dst = s_sum[:, rc * (RC // P):(rc + 1) * (RC // P), :]
                        if s == 0:
                            nc.vector.tensor_copy(dst, rt[:])
                        else:
                            nc.vector.tensor_tensor(out=dst, in0=dst, in1=rt[:],
                                                    op=Alu.add)

                # combine (per-block broadcast of -2/deg)
                if k == 1:
                    # z1 = -dinv2 * S = 0.5 * n2d2 * S
                    for j in range(NBLK):
                        nc.vector.tensor_tensor(
                            out=z_cur[:, j, :], in0=s_sum[:, j, :],
                            in1=n2d2_t[:, j:j + 1].to_broadcast([P, F]),
                            op=Alu.mult)
                    nc.vector.tensor_scalar_mul(z_cur[:], z_cur[:], 0.5)
                    z_new = z_cur
                else:
                    # z_next = n2d2*S - z_prev  (write into z_prev slot)
                    for j in range(NBLK):
                        nc.vector.tensor_tensor(
                            out=s_sum[:, j, :], in0=s_sum[:, j, :],
                            in1=n2d2_t[:, j:j + 1].to_broadcast([P, F]),
                            op=Alu.mult)
                    nc.vector.tensor_tensor(out=z_prev[:], in0=s_sum[:],
                                            in1=z_prev[:], op=Alu.subtract)
                    z_new = z_prev
                    z_prev, z_cur = z_cur, z_new
                # out_acc += coe[k] * z_new   (reuse s_sum as scratch)
                nc.vector.tensor_scalar_mul(s_sum[:], z_new[:], float(coe[k]))
                nc.vector.tensor_tensor(out=out_acc[:], in0=out_acc[:],
                                        in1=s_sum[:], op=Alu.add)

            # final scale by sqrt(deg) and store
            for j in range(NBLK):
                nc.vector.tensor_tensor(
                    out=out_acc[:, j, :], in0=out_acc[:, j, :],
                    in1=sqd_t[:, j:j + 1].to_broadcast([P, F]), op=Alu.mult)
            if STAGE == "nop3":
                nc.sync.dma_start(out_d[0:1, :], out_acc[0:1, 0, :])
            else:
                nc.sync.dma_start(
                    out_d[0:12416, :].rearrange("(j p) f -> p j f", p=P),
                    out_acc[:, 0:97, :])
                nc.sync.dma_start(
                    out_d[12416:RPC, :].rearrange("(j p) f -> p j f", p=84),
                    out_acc[0:84, 97:98, :])

    t0 = time.time()
    nc.compile()
    print(f"bacc compile: {time.time() - t0:.1f}s", flush=True)
    return nc


def prepare(inputs):
    """Host preprocessing + program build. Returns (nc, in_maps)."""
    feature = np.asarray(inputs["feature"], np.float32)
    W1 = np.asarray(inputs["W1"], np.float32)
    b1 = np.asarray(inputs["b1"], np.float32)
    W2 = np.asarray(inputs["W2"], np.float32)
    b2 = np.asarray(inputs["b2"], np.float32)
    temp = np.asarray(inputs["temp"], np.float32)
    edge_index = np.asarray(inputs["edge_index"])

    # Chebyshev coefficients (host, tiny)
    coe_tmp = np.maximum(temp, 0.0)
    j = np.arange(K + 1, dtype=np.float64)
    theta = (K - j + 0.5) * np.pi / (K + 1)
    i = np.arange(K + 1, dtype=np.float64)
    T = np.cos(i[:, None] * theta[None, :])
    coe = ((2.0 / (K + 1)) * (T @ coe_tmp.astype(np.float64))).astype(np.float32)

    degs, deg, pi, inv_pi, S_sched, streams, T_s = _prep(edge_index)

    degf = deg.astype(np.float32)
    dinv = np.where(deg > 0, 1.0 / np.sqrt(np.maximum(degf, 1.0)), 1.0).astype(np.float32)
    n2d2 = np.where(deg > 0, -2.0 / np.maximum(degf, 1.0), -2.0).astype(np.float32)
    sqd = np.where(deg > 0, np.sqrt(np.maximum(degf, 1.0)), 1.0).astype(np.float32)

    def rowmajor(vec_c):  # [RPC] -> [128, NBLK] with r = 128*j + p
        v = np.zeros(RPC_PAD, np.float32)
        v[:RPC] = vec_c
        return np.ascontiguousarray(v.reshape(NBLK, P).T)

    in_maps = []
    for c in range(NCORES):
        fT = np.zeros((NFEAT, RPC_PAD), ml_dtypes.bfloat16)
        fT[:, :RPC] = feature[c * RPC:(c + 1) * RPC].T.astype(ml_dtypes.bfloat16)
        m = {
            "featT": fT,
            "W1T": np.ascontiguousarray(W1.T),
            "b1t": np.ascontiguousarray(b1.reshape(2, P).T),
            "W2T": np.ascontiguousarray(W2.T),
            "b2t": np.ascontiguousarray(b2.reshape(F, 1)),
            "dinv": rowmajor(dinv[c * RPC:(c + 1) * RPC]),
            "n2d2": rowmajor(n2d2[c * RPC:(c + 1) * RPC]),
            "sqd": rowmajor(sqd[c * RPC:(c + 1) * RPC]),
        }
        for s in range(4):
            m[f"idx{s}"] = _wrap_idx(streams[c][s])
            r = np.full(RPC_PAD, -1, np.int16)
            r[:RPC] = inv_pi[c, s]
            m[f"ridx{s}"] = _wrap_idx(r)
        if STAGE in ("nop2", "nop3"):
            m["featT"] = np.zeros((1, 1), ml_dtypes.bfloat16)
            for s in range(4):
                m[f"idx{s}"] = np.zeros((1, 1), np.int16)
                m[f"ridx{s}"] = np.zeros((1, 1), np.int16)
        in_maps.append(m)

    t0 = time.time()
    nc = _build_program(S_sched, T_s, coe)
    print(f"build+compile total: {time.time() - t0:.1f}s", flush=True)
    return nc, in_maps


def assemble(results):
    out = np.empty((N, F), np.float32)
    for c in range(NCORES):
        out[c * RPC:(c + 1) * RPC] = results[c]["out"]
    return out


def kernel(**inputs):
    from concourse.bass_utils import run_bass_kernel_spmd

    nc, in_maps = prepare(inputs)
    t0 = time.time()
    res = run_bass_kernel_spmd(nc, in_maps, list(range(NCORES)), trace=TRACE)
    print(f"neff compile+run: {time.time() - t0:.1f}s", flush=True)
    LAST["exec_time_ns"] = res.exec_time_ns
    LAST["profile_json"] = res.profile_json
    return assemble(res.results)



# revision 38
# speedup vs baseline: 1.4078x; 1.0320x over previous
"""ChebNetII distributed Trainium2 kernel (8 NeuronCores).

Strategy:
  * Rows (nodes) sharded 12500/core. MLP computed on-device per core in
    bf16 (fp32 PSUM accumulation).
  * Chebyshev propagation in "z-space": z = D^-1/2 Tx, so the per-edge
    weight is exactly 1 (pure adjacency gather+sum) and the D scaling is a
    per-row multiply:  z_{k+1} = -2 deg^-1 * S(z_k) - z_{k-1},
    where S(z)[r] = sum_{edges (r,c)} z[c].
  * Per prop step the full z table (bf16, node pairs packed into 256B rows)
    is AllGathered into a SHARED-address-space DRAM table (one physical
    copy per chip; makes the collective ~free vs per-core Local copies).
  * Each core bulk dma_gathers its edges' source pairs (4 streams =
    2 index windows x 2 node parities, int16 index limit) in 512-descriptor
    calls round-robined over 4 SWDGE queues (512 descs = 32/engine keeps
    the descriptor ring un-congested; 1024-desc calls hit a throughput
    cliff), and reduces slots into rows with identity-matmul PSUM
    accumulation over a degree-sorted slot schedule. Partials are realigned
    back to canonical row order with small dma_gathers.
"""
import os
import sys
import time

sys.path.insert(0, "/opt/trn_rl_repo")

import numpy as np
import ml_dtypes

K_RUN = 10
STAGE = "full"
TRACE = False                                    # set by test.py for profiling
LAST = {}                                        # exec_time_ns etc. for test.py

N = 100000
K = 10
F = 64
NFEAT, NHID = 512, 256
NCORES = 8
RPC = 12500            # rows per core
RPC_PAD = 12544        # 98*128
NBLK = RPC_PAD // 128  # 98
PAIRS_PC = RPC_PAD // 2          # 6272
TBL_PAIRS = NCORES * PAIRS_PC    # 50176
WIN_PAIRS = TBL_PAIRS // 2       # 25088
ZERO_IDX = 6250        # local pair idx of a guaranteed-zero pair (first pad pair of window's first core block)
P = 128
BG = 1                 # row-blocks per psum group
GROUP = P * BG         # 128
NGROUPS = 100          # 12800 sorted rows
ROWS_SORT_PAD = NGROUPS * GROUP
GCH = 512              # idxs per main dma_gather call (SWDGE ring is ~256 descs/engine)
SECPC = GCH // GROUP   # sections per gather call (4)
SLAB = 32              # idx sections per slab load
RC = 896               # combine chunk rows (=7*128); 12544/896 = 14 chunks
ACC_ROWS = 13056       # 12544 canonical + 512 dummy rows for sorted-pad slots


def _prep(edge_index):
    row = edge_index[0].astype(np.int64)
    col = edge_index[1].astype(np.int64)

    deg = np.bincount(row, minlength=N).astype(np.int64)

    q_t = PAIRS_PC * (col // RPC) + (col % RPC) // 2
    w = q_t // WIN_PAIRS
    lidx = (q_t % WIN_PAIRS).astype(np.int64)
    par = col % 2
    s_of_e = 2 * w + par
    core = row // RPC
    lr = row % RPC

    key = (core * 4 + s_of_e) * RPC + lr
    order = np.argsort(key, kind="stable")
    core_s, s_s, lr_s, lidx_s = core[order], s_of_e[order], lr[order], lidx[order]
    kk = key[order]

    degs = np.bincount(kk, minlength=NCORES * 4 * RPC).reshape(NCORES, 4, RPC)

    pi = np.zeros((NCORES, 4, RPC), np.int64)
    inv_pi = np.zeros((NCORES, 4, RPC), np.int64)
    S_cs = np.zeros((NCORES, 4, NGROUPS), np.int64)
    for c in range(NCORES):
        for si in range(4):
            o = np.argsort(-degs[c, si], kind="stable")
            pi[c, si] = o
            inv_pi[c, si, o] = np.arange(RPC)
            d_pad = np.zeros(ROWS_SORT_PAD, np.int64)
            d_pad[:RPC] = degs[c, si, o]
            S_cs[c, si] = d_pad.reshape(NGROUPS, GROUP).max(1)
    S_sched = S_cs.max(axis=0)          # [4, NGROUPS]
    T_s = [int(GROUP * S_sched[si].sum()) for si in range(4)]
    cumS = [np.concatenate([[0], np.cumsum(S_sched[si])]) for si in range(4)]

    # slot position of each edge within its stream
    first = np.ones(len(kk), bool)
    first[1:] = kk[1:] != kk[:-1]
    seg_ids = np.cumsum(first) - 1
    starts = np.flatnonzero(first)
    m_in_row = np.arange(len(kk)) - starts[seg_ids]

    streams = [[np.full(T_s[si], ZERO_IDX, np.int16) for si in range(4)]
               for _ in range(NCORES)]
    for c in range(NCORES):
        msk_c = core_s == c
        for si in range(4):
            msk = msk_c & (s_s == si)
            pos = inv_pi[c, si, lr_s[msk]]
            g = pos // GROUP
            b = pos % GROUP
            off = GROUP * cumS[si][g] + GROUP * m_in_row[msk] + b
            streams[c][si][off] = lidx_s[msk].astype(np.int16)

    return degs, deg, pi, inv_pi, S_sched, streams, T_s


def _wrap_idx(idx_flat):
    """[n] -> [128, n/16] wrapped (i -> (i%16, i//16)) + replicated x8."""
    n = len(idx_flat)
    assert n % 16 == 0
    a = idx_flat.reshape(n // 16, 16).T  # [16, n/16]
    return np.ascontiguousarray(np.tile(a, (8, 1)))


def _build_program(S_sched, T_s, coe):
    import concourse.bass as bass
    import concourse.tile as tile
    from concourse import bacc, mybir
    from concourse.library_config import mlp as mlp_lib

    dt = mybir.dt
    Alu = mybir.AluOpType
    Act = mybir.ActivationFunctionType

    nc = bacc.Bacc("TRN2", target_bir_lowering=False, debug=False,
                   num_devices=NCORES, num_swdge_queues=4,
                   dynamic_dma_scratch_size=32768)

    lite = STAGE in ("nop2", "nop3")   # tiny inputs: measures pure exec/transfer floor
    if lite:
        featT = nc.dram_tensor("featT", [1, 1], dt.float32, kind="ExternalInput")
        idx_d = [nc.dram_tensor(f"idx{s}", [1, 1], dt.int16,
                                kind="ExternalInput") for s in range(4)]
        ridx_d = [nc.dram_tensor(f"ridx{s}", [1, 1], dt.int16,
                                 kind="ExternalInput") for s in range(4)]
    else:
        featT = nc.dram_tensor("featT", [NFEAT, RPC_PAD], dt.bfloat16, kind="ExternalInput")
        idx_d = [nc.dram_tensor(f"idx{s}", [P, T_s[s] // 16], dt.int16,
                                kind="ExternalInput") for s in range(4)]
        ridx_d = [nc.dram_tensor(f"ridx{s}", [P, RPC_PAD // 16], dt.int16,
                                 kind="ExternalInput") for s in range(4)]
    f16 = dt.float16
    W1T = nc.dram_tensor("W1T", [NFEAT, NHID], dt.float32, kind="ExternalInput")
    b1t_d = nc.dram_tensor("b1t", [P, 2], dt.float32, kind="ExternalInput")
    W2T = nc.dram_tensor("W2T", [NHID, F], dt.float32, kind="ExternalInput")
    b2t_d = nc.dram_tensor("b2t", [F, 1], dt.float32, kind="ExternalInput")
    dinv_d = nc.dram_tensor("dinv", [P, NBLK], dt.float32, kind="ExternalInput")
    n2d2_d = nc.dram_tensor("n2d2", [P, NBLK], dt.float32, kind="ExternalInput")
    sqd_d = nc.dram_tensor("sqd", [P, NBLK], dt.float32, kind="ExternalInput")
    if STAGE == "nop3":
        out_d = nc.dram_tensor("out", [1, F], dt.float32, kind="ExternalOutput")
    else:
        out_d = nc.dram_tensor("out", [RPC, F], dt.float32, kind="ExternalOutput")

    # section lists per stream: [(g, m, is_last)]
    sections = []
    for s in range(4):
        sec = []
        for g in range(NGROUPS):
            Sg = int(S_sched[s][g])
            for m in range(Sg):
                sec.append((g, m, m == Sg - 1))
        sections.append(sec)

    with tile.TileContext(nc) as tc:
        with (
            tc.tile_pool(name="dram", bufs=1, space="DRAM") as dram,
            tc.tile_pool(name="consts", bufs=1) as consts,
            tc.tile_pool(name="zs", bufs=1) as zs,
            tc.tile_pool(name="mlp", bufs=2) as mlppool,
            tc.tile_pool(name="gp", bufs=16) as gpool,
            tc.tile_pool(name="ip", bufs=8) as ipool,
            tc.tile_pool(name="ev", bufs=8) as evpool,
            tc.tile_pool(name="rt", bufs=4) as rtpool,
            tc.tile_pool(name="ps1", bufs=2, space="PSUM") as ps1,
            tc.tile_pool(name="ps2", bufs=1, space="PSUM") as ps2,
            tc.tile_pool(name="psT", bufs=1, space="PSUM") as psT,
            tc.tile_pool(name="psG", bufs=4, space="PSUM") as psG,
        ):
            nc.gpsimd.load_library(mlp_lib)

            contribs = [dram.tile([RPC_PAD, F], f16, name=f"contrib{k}")
                        for k in range(K)]
            tables = [dram.tile([TBL_PAIRS, 2 * F], f16,
                                name=f"table{k}", addr_space="Shared")
                      for k in range(K)]
            halftables = [dram.tile([TBL_PAIRS // 2, 2 * F], f16,
                                    name=f"htable{k}", addr_space="Shared")
                          for k in range(K)] if STAGE == "gohalfcc" else None
            partials = [dram.tile([ROWS_SORT_PAD, F], dt.float32, name=f"partial{s}")
                        for s in range(4)]

            # ---- constants ----
            iota_p = consts.tile([P, 1], dt.int32)
            nc.gpsimd.iota(iota_p[:], pattern=[[0, 1]], base=0, channel_multiplier=1)
            iota_pf = consts.tile([P, 1], dt.float32)
            nc.vector.tensor_copy(iota_pf[:], iota_p[:])
            iota_f = consts.tile([P, P], dt.int32)
            nc.gpsimd.iota(iota_f[:], pattern=[[1, P]], base=0, channel_multiplier=0)
            iota_ff = consts.tile([P, P], dt.float32)
            nc.vector.tensor_copy(iota_ff[:], iota_f[:])
            ident_bf = consts.tile([P, P], f16)
            nc.vector.tensor_tensor(out=ident_bf[:], in0=iota_ff[:],
                                    in1=iota_pf[:].to_broadcast([P, P]),
                                    op=Alu.is_equal)
            ident64 = consts.tile([F, F], dt.float32)
            nc.vector.tensor_tensor(out=ident64[:], in0=iota_ff[:F, :F],
                                    in1=iota_pf[:F, :].to_broadcast([F, F]),
                                    op=Alu.is_equal)

            w1 = consts.tile([P, 4, NHID], dt.float32)
            nc.sync.dma_start(w1[:], W1T[:, :].rearrange("(k p) h -> p k h", p=P))
            w2 = consts.tile([P, 2, F], dt.float32)
            nc.sync.dma_start(w2[:], W2T[:, :].rearrange("(k p) h -> p k h", p=P))
            w1b = consts.tile([P, 4, NHID], dt.bfloat16)
            nc.vector.tensor_copy(w1b[:], w1[:])
            w2b = consts.tile([P, 2, F], dt.bfloat16)
            nc.vector.tensor_copy(w2b[:], w2[:])
            b1tt = consts.tile([P, 2], dt.float32)
            nc.sync.dma_start(b1tt[:], b1t_d[:, :])
            b2tt = consts.tile([F, 1], dt.float32)
            nc.sync.dma_start(b2tt[:], b2t_d[:, :])
            dinv_t = consts.tile([P, NBLK], dt.float32)
            nc.sync.dma_start(dinv_t[:], dinv_d[:, :])
            n2d2_t = consts.tile([P, NBLK], dt.float32)
            nc.sync.dma_start(n2d2_t[:], n2d2_d[:, :])
            sqd_t = consts.tile([P, NBLK], dt.float32)
            nc.sync.dma_start(sqd_t[:], sqd_d[:, :])

            ridx_t = []
            for si in range(4):
                rtile = consts.tile([P, RPC_PAD // 16], dt.int16,
                                    name=f"ridx_t{si}")
                if STAGE not in ("nop2", "nop3"):
                    nc.sync.dma_start(rtile[:], ridx_d[si][:, :])
                ridx_t.append(rtile)

            # zero the contrib pad rows once (rows RPC..RPC_PAD)
            zpad = consts.tile([44, F], f16)
            nc.vector.memset(zpad[:], 0.0)
            for k in range(K):
                nc.sync.dma_start(contribs[k][RPC:RPC_PAD, :], zpad[:])

            # ---- persistent state ----
            zA = zs.tile([P, NBLK, F], f16)
            zB = zs.tile([P, NBLK, F], f16)
            out_acc = zs.tile([P, NBLK, F], dt.float32)
            s_sum = zs.tile([P, NBLK, F], dt.float32)

            # ---- MLP -> z0 (into zA) ----
            chunks = [(i * 512, 512) for i in range(24)] + [(24 * 512, 256)]
            if STAGE in ("nop", "nop2", "nop3"):
                nc.vector.memset(zA[:], 0.0)
                chunks = []
            for (c0, C) in chunks:
                ft = mlppool.tile([P, 4, 512], dt.bfloat16, tag="featT", bufs=2)
                nc.sync.dma_start(
                    ft[:, :, :C],
                    featT[:, c0:c0 + C].rearrange("(k p) c -> p k c", p=P))
                x1h = []
                for h in range(2):
                    pm = ps1.tile([P, 512], dt.float32, space="PSUM", tag="ps1")
                    for k in range(4):
                        nc.tensor.matmul(out=pm[:, :C],
                                         lhsT=w1b[:, k, 128 * h:128 * (h + 1)],
                                         rhs=ft[:, k, :C],
                                         start=(k == 0), stop=(k == 3))
                    xh = mlppool.tile([P, 512], dt.bfloat16, tag="x1")
                    nc.scalar.activation(xh[:, :C], pm[:, :C], Act.Relu,
                                         bias=b1tt[:, h:h + 1])
                    x1h.append(xh)
                pm2 = ps2.tile([F, 512], dt.float32, space="PSUM", tag="ps2")
                for h in range(2):
                    nc.tensor.matmul(out=pm2[:, :C], lhsT=w2b[:, h, :],
                                     rhs=x1h[h][:, :C],
                                     start=(h == 0), stop=(h == 1))
                x2 = mlppool.tile([F, 512], dt.float32, tag="x2")
                nc.scalar.activation(x2[:, :C], pm2[:, :C], Act.Identity,
                                     bias=b2tt[:, 0:1])
                for jj in range(C // 128):
                    jb = c0 // 128 + jj
                    pt = psT.tile([P, F], dt.float32, space="PSUM", tag="psT")
                    nc.tensor.transpose(pt[:], x2[:, 128 * jj:128 * (jj + 1)],
                                        ident64[:])
                    nc.vector.tensor_tensor(
                        out=zA[:, jb, :], in0=pt[:],
                        in1=dinv_t[:, jb:jb + 1].to_broadcast([P, F]),
                        op=Alu.mult)

            # out_acc = coe0/2 * z0
            nc.vector.tensor_scalar_mul(out_acc[:], zA[:], float(coe[0]) / 2.0)

            z_prev, z_cur = zA, zB
            for k in range(1, K_RUN + 1):
                if STAGE in ("mlponly", "nop", "nop2", "nop3"):
                    continue
                zsrc = z_prev if (k == 1 or STAGE in ("mlp", "pub", "gather", "gatheronly", "gonocc", "gohalfcc")) else z_cur
                # publish z_{k-1}: cast-DMA into contrib then AllGather
                contrib = contribs[k - 1]
                table = tables[k - 1]
                nc.sync.dma_start(
                    contrib[0:12416, :].rearrange("(j p) f -> p j f", p=P),
                    zsrc[:, 0:97, :])
                nc.sync.dma_start(
                    contrib[12416:RPC, :].rearrange("(j p) f -> p j f", p=84),
                    zsrc[0:84, 97:98, :])
                if STAGE == "pub":
                    continue
                if STAGE == "gonocc":
                    pass
                elif STAGE == "gohalfcc":
                    nc.gpsimd.collective_compute(
                        "AllGather", Alu.bypass,
                        replica_groups=[list(range(NCORES))],
                        ins=[contrib[0:RPC_PAD // 2, :].opt()],
                        outs=[halftables[k - 1][:].opt()])
                else:
                    nc.gpsimd.collective_compute(
                        "AllGather", Alu.bypass,
                        replica_groups=[list(range(NCORES))],
                        ins=[contrib[:].opt()], outs=[table[:].opt()])

                if STAGE == "mlp":
                    continue
                # gather + identity-matmul reduce, per stream.
                # 512-desc dma_gather calls (ring stays <=32 descs/engine),
                # round-robin over 4 SWDGE queues; TWO gathers share one idx
                # load + one gt tile + one PE matmul over 1024 slots.
                # interleave the 4 streams round-robin, one SWDGE queue per
                # stream; each 512-idx dma_gather covers SECPC consecutive
                # 128-row sections of its stream.
                psum_maps = [{} for _ in range(4)]
                islabs = [None] * 4
                ncalls = [(len(sections[s]) + SECPC - 1) // SECPC
                          for s in range(4)]
                for cix in range(max(ncalls)):
                    for s in range(4):
                        if cix >= ncalls[s]:
                            continue
                        t0 = cix * SECPC
                        nsec = min(SECPC, len(sections[s]) - t0)
                        win = s // 2
                        par = s % 2
                        src = table[win * WIN_PAIRS:(win + 1) * WIN_PAIRS, :]
                        i0 = t0 * GROUP
                        sl = t0 % SLAB
                        if sl == 0:
                            ns = min(SLAB, len(sections[s]) - t0)
                            islabs[s] = ipool.tile([P, SLAB * GROUP // 16],
                                                   dt.int16, tag=f"idxslab{s}",
                                                   name=f"islab{s}")
                            nc.sync.dma_start(
                                islabs[s][:, :ns * GROUP // 16],
                                idx_d[s][:, i0 // 16:(i0 + ns * GROUP) // 16])
                        it = islabs[s][:, sl * GROUP // 16:
                                       (sl + nsec) * GROUP // 16]
                        gt = gpool.tile([P, SECPC * BG, 2 * F], f16, tag="g")
                        nc.gpsimd.dma_gather(
                            gt[:, :nsec * BG, :], src, it, nsec * GROUP,
                            nsec * GROUP, 2 * F, elem_step=2 * F, queue_num=s)
                        for j in range(nsec):
                            g, m, last = sections[s][t0 + j]
                            if m == 0:
                                psum_maps[s][g] = psG.tile(
                                    [P, BG, F], dt.float32, space="PSUM",
                                    tag="psG", name=f"pg_{k}_{s}_{g}")
                            pm = psum_maps[s][g]
                            nc.tensor.matmul(
                                out=pm[:],
                                lhsT=ident_bf[:],
                                rhs=gt[:, j * BG:(j + 1) * BG,
                                       par * F:(par + 1) * F],
                                start=(m == 0), stop=last)
                            if last:
                                ev = evpool.tile([P, BG, F], dt.float32,
                                                 tag="ev")
                                nc.vector.tensor_copy(ev[:], pm[:])
                                nc.sync.dma_start(
                                    partials[s][GROUP * g:GROUP * (g + 1), :]
                                    .rearrange("(b p) f -> p b f", p=P),
                                    ev[:])

                if STAGE in ("gather", "gatheronly", "gonocc", "gohalfcc"):
                    continue
                # realign partials into s_sum
                for rc in range(RPC_PAD // RC):
                    for s in range(4):
                        reg = max(0, min(RPC - rc * RC, RC))
                        rt = rtpool.tile([P, RC // P, F], dt.float32, tag="rt")
                        nc.gpsimd.dma_gather(
                            rt[:], partials[s][:, :],
                            ridx_t[s][:, rc * (RC // 16):(rc + 1) * (RC // 16)],
                            RC, reg, F,
                            elem_step=F, queue_num=(s + rc) % 4)
                        dst = s_sum[:, rc * (RC // P):(rc + 1) * (RC // P), :]
                        if s == 0:
                            nc.vector.tensor_copy(dst, rt[:])
                        else:
                            nc.vector.tensor_tensor(out=dst, in0=dst, in1=rt[:],
                                                    op=Alu.add)

                # combine (per-block broadcast of -2/deg)
                if k == 1:
                    # z1 = -dinv2 * S = 0.5 * n2d2 * S
                    for j in range(NBLK):
                        nc.vector.tensor_tensor(
                            out=z_cur[:, j, :], in0=s_sum[:, j, :],
                            in1=n2d2_t[:, j:j + 1].to_broadcast([P, F]),
                            op=Alu.mult)
                    nc.vector.tensor_scalar_mul(z_cur[:], z_cur[:], 0.5)
                    z_new = z_cur
                else:
                    # z_next = n2d2*S - z_prev  (write into z_prev slot)
                    for j in range(NBLK):
                        nc.vector.tensor_tensor(
                            out=s_sum[:, j, :], in0=s_sum[:, j, :],
                            in1=n2d2_t[:, j:j + 1].to_broadcast([P, F]),
                            op=Alu.mult)
                    nc.vector.tensor_tensor(out=z_prev[:], in0=s_sum[:],
                                            in1=z_prev[:], op=Alu.subtract)
                    z_new = z_prev
                    z_prev, z_cur = z_cur, z_new
                # out_acc += coe[k] * z_new   (reuse s_sum as scratch)
                nc.vector.tensor_scalar_mul(s_sum[:], z_new[:], float(coe[k]))
                nc.vector.tensor_tensor(out=out_acc[:], in0=out_acc[:],
                                        in1=s_sum[:], op=Alu.add)

            # final scale by sqrt(deg) and store
            for j in range(NBLK):
                nc.vector.tensor_tensor(
                    out=out_acc[:, j, :], in0=out_acc[:, j, :],
                    in1=sqd_t[:, j:j + 1].to_broadcast([P, F]), op=Alu.mult)
            if STAGE == "nop3":
                nc.sync.dma_start(out_d[0:1, :], out_acc[0:1, 0, :])
            else:
                nc.sync.dma_start(
                    out_d[0:12416, :].rearrange("(j p) f -> p j f", p=P),
                    out_acc[:, 0:97, :])
                nc.sync.dma_start(
                    out_d[12416:RPC, :].rearrange("(j p) f -> p j f", p=84),
                    out_acc[0:84, 97:98, :])

    t0 = time.time()
    nc.compile()
    print(f"bacc compile: {time.time() - t0:.1f}s", flush=True)
    return nc


def prepare(inputs):
    """Host preprocessing + program build. Returns (nc, in_maps)."""
    feature = np.asarray(inputs["feature"], np.float32)
    W1 = np.asarray(inputs["W1"], np.float32)
    b1 = np.asarray(inputs["b1"], np.float32)
    W2 = np.asarray(inputs["W2"], np.float32)
    b2 = np.asarray(inputs["b2"], np.float32)
    temp = np.asarray(inputs["temp"], np.float32)
    edge_index = np.asarray(inputs["edge_index"])

    # Chebyshev coefficients (host, tiny)
    coe_tmp = np.maximum(temp, 0.0)
    j = np.arange(K + 1, dtype=np.float64)
    theta = (K - j + 0.5) * np.pi / (K + 1)
    i = np.arange(K + 1, dtype=np.float64)
    T = np.cos(i[:, None] * theta[None, :])
    coe = ((2.0 / (K + 1)) * (T @ coe_tmp.astype(np.float64))).astype(np.float32)

    degs, deg, pi, inv_pi, S_sched, streams, T_s = _prep(edge_index)

    degf = deg.astype(np.float32)
    dinv = np.where(deg > 0, 1.0 / np.sqrt(np.maximum(degf, 1.0)), 1.0).astype(np.float32)
    n2d2 = np.where(deg > 0, -2.0 / np.maximum(degf, 1.0), -2.0).astype(np.float32)
    sqd = np.where(deg > 0, np.sqrt(np.maximum(degf, 1.0)), 1.0).astype(np.float32)

    def rowmajor(vec_c):  # [RPC] -> [128, NBLK] with r = 128*j + p
        v = np.zeros(RPC_PAD, np.float32)
        v[:RPC] = vec_c
        return np.ascontiguousarray(v.reshape(NBLK, P).T)

    in_maps = []
    for c in range(NCORES):
        fT = np.zeros((NFEAT, RPC_PAD), ml_dtypes.bfloat16)
        fT[:, :RPC] = feature[c * RPC:(c + 1) * RPC].T.astype(ml_dtypes.bfloat16)
        m = {
            "featT": fT,
            "W1T": np.ascontiguousarray(W1.T),
            "b1t": np.ascontiguousarray(b1.reshape(2, P).T),
            "W2T": np.ascontiguousarray(W2.T),
            "b2t": np.ascontiguousarray(b2.reshape(F, 1)),
            "dinv": rowmajor(dinv[c * RPC:(c + 1) * RPC]),
            "n2d2": rowmajor(n2d2[c * RPC:(c + 1) * RPC]),
            "sqd": rowmajor(sqd[c * RPC:(c + 1) * RPC]),
        }
        for s in range(4):
            m[f"idx{s}"] = _wrap_idx(streams[c][s])
            r = np.full(RPC_PAD, -1, np.int16)
            r[:RPC] = inv_pi[c, s]
            m[f"ridx{s}"] = _wrap_idx(r)
        if STAGE in ("nop2", "nop3"):
            m["featT"] = np.zeros((1, 1), ml_dtypes.bfloat16)
            for s in range(4):
                m[f"idx{s}"] = np.zeros((1, 1), np.int16)
                m[f"ridx{s}"] = np.zeros((1, 1), np.int16)
        in_maps.append(m)

    t0 = time.time()
    nc = _build_program(S_sched, T_s, coe)
    print(f"build+compile total: {time.time() - t0:.1f}s", flush=True)
    return nc, in_maps


def assemble(results):
    out = np.empty((N, F), np.float32)
    for c in range(NCORES):
        out[c * RPC:(c + 1) * RPC] = results[c]["out"]
    return out


def kernel(**inputs):
    from concourse.bass_utils import run_bass_kernel_spmd

    nc, in_maps = prepare(inputs)
    t0 = time.time()
    res = run_bass_kernel_spmd(nc, in_maps, list(range(NCORES)), trace=TRACE)
    print(f"neff compile+run: {time.time() - t0:.1f}s", flush=True)
    LAST["exec_time_ns"] = res.exec_time_ns
    LAST["profile_json"] = res.profile_json
    return assemble(res.results)

